# revision 8
# baseline (speedup 1.0000x reference)
"""AttentionBlock Trainium2 kernel: GroupNorm -> QKV -> MHA -> proj -> residual.

Data-parallel over batch B=8 across 8 NeuronCores (one batch image per core).
QKV/proj/score matmuls run in bf16 (fp32 PSUM accumulation); the
attention-value matmuls run in fp8 e4m3 DoubleRow (2 k-chunks per pass):
E is stored as exp(s*scale)/16 in e4m3 (the /16 keeps e4m3 in range; it
rides both the numerator and the ones-column denominator so it cancels).

Per-core layouts (C=512 channels, HW=1024 tokens, 8 heads, hd=64):
  x            [C, HW] bf16 (stats + residual tolerate bf16)
  xn           [C, HW] bf16, per-128-channel-chunk GroupNorm
  q, k         qk_sb[128, 8 oc, 1024] bf16; pair hp: oc=hp (q), 4+hp (k),
               head a at partitions a*64:(a+1)*64
  vT (fp8)     vt8[128, kcp 4, ko 2, head 8, 68] (65 used: 64 + ones col)
  scoresT      [k, q] PSUM [128, 1024] per (head, kc)
  E            [128, 2 head, 8 kc, 1024] fp8 per pair
  att          [C, HW] bf16; out [C, HW] fp32 = proj(att) + pb + x

PSUM (8 banks): ps_s x2 (scores / lead-in qk / proj py0-1) = 4 banks;
avA x1 (in-pair AV accumulator / py2) + avB x1 (rotating: GN stats, V^T,
mid-pair qk, deferred a=0 AV, py3) = 4 banks.

Softmax denominators: AV psum row 64 -> SBUF; reciprocal_approx_fast on the
row; DRAM bounce broadcast to [64, 1024]; multiply (partition-shifted write
for the odd head).
"""

import sys

if "/opt/trn_rl_repo" not in sys.path:
    sys.path.insert(0, "/opt/trn_rl_repo")

import numpy as np
import ml_dtypes

import concourse.bass as bass
import concourse.tile as tile
from concourse import mybir, bacc
from concourse.bass_utils import run_bass_kernel_spmd

AF = mybir.ActivationFunctionType
ALU = mybir.AluOpType
F32 = mybir.dt.float32
BF16 = mybir.dt.bfloat16
FP8 = mybir.dt.float8e4
DR = mybir.MatmulPerfMode.DoubleRow

C = 512
HW = 1024
NHEADS = 8
HD = 64
HDP = 68  # padded head stride so the DoubleRow ko-step (8*HDP) is 16-aligned
NGROUPS = 32
GSIZE = 16
EPS = 1e-5
SCALE = HD ** -0.5
EBIAS = -4.0 * float(np.log(2.0))  # exp output scaled by 1/16 for e4m3 range
CC = 4
OCQK = 8
QC = 2


def _build():
    nc = bacc.Bacc("TRN2", target_bir_lowering=False, debug=False, num_devices=8)

    x_d = nc.dram_tensor("x", [C, HW], BF16, kind="ExternalInput")
    qw_d = nc.dram_tensor("qw", [C, 3 * C], BF16, kind="ExternalInput")
    pw_d = nc.dram_tensor("pw", [C, C], BF16, kind="ExternalInput")
    qkb_d = nc.dram_tensor("qkb", [128, 8], F32, kind="ExternalInput")
    vbb_d = nc.dram_tensor("vbb", [128, C], F32, kind="ExternalInput")
    pb_d = nc.dram_tensor("pb", [128, 4], F32, kind="ExternalInput")
    gnw_d = nc.dram_tensor("gnw", [128, 4], F32, kind="ExternalInput")
    gnb_d = nc.dram_tensor("gnb", [128, 4], F32, kind="ExternalInput")
    ind_d = nc.dram_tensor("ind", [128, 8], F32, kind="ExternalInput")
    indt_d = nc.dram_tensor("indt", [8, 128], F32, kind="ExternalInput")
    out_d = nc.dram_tensor("out", [C, HW], F32, kind="ExternalOutput")

    with tile.TileContext(nc) as tc:
        with (
            tc.tile_pool(name="consts", bufs=1) as consts,
            tc.tile_pool(name="epool", bufs=2) as epool,
            tc.tile_pool(name="small", bufs=4) as small,
            tc.tile_pool(name="outp", bufs=3) as outp,
            tc.tile_pool(name="drp", bufs=4, space="DRAM") as drp,
            tc.tile_pool(name="ps_s", bufs=2, space="PSUM") as ps_s,
            tc.tile_pool(name="ps_av", bufs=1, space="PSUM") as ps_av,
        ):
            # ---- input DMAs: tiny tables first, then x chunks, then weights ----
            ind = consts.tile([128, 8], F32, tag="ind")
            nc.sync.dma_start(out=ind, in_=ind_d.ap())
            indt = consts.tile([8, 128], F32, tag="indt")
            nc.sync.dma_start(out=indt, in_=indt_d.ap())
            gnw = consts.tile([128, 4], F32, tag="gnw")
            nc.sync.dma_start(out=gnw, in_=gnw_d.ap())
            gnb = consts.tile([128, 4], F32, tag="gnb")
            nc.sync.dma_start(out=gnb, in_=gnb_d.ap())
            qkb = consts.tile([128, 8], F32, tag="qkb")
            nc.sync.dma_start(out=qkb, in_=qkb_d.ap())
            vbb = consts.tile([128, C], F32, tag="vbb")
            nc.sync.dma_start(out=vbb, in_=vbb_d.ap())
            pb = consts.tile([128, 4], F32, tag="pb")
            nc.sync.dma_start(out=pb, in_=pb_d.ap())
            x_sb = consts.tile([128, CC, HW], BF16, tag="x")
            x_r = x_d.ap().rearrange("(cc p) hw -> p cc hw", p=128)
            for cc in range(CC):
                nc.sync.dma_start(out=x_sb[:, cc, :], in_=x_r[:, cc, :])
            qw_sb = consts.tile([128, CC, 3 * C], BF16, tag="qw")
            qw_r = qw_d.ap().rearrange("(cc p) o -> p cc o", p=128)
            nc.sync.dma_start(out=qw_sb[:, :, 0:2 * C], in_=qw_r[:, :, 0:2 * C])
            nc.sync.dma_start(out=qw_sb[:, :, 2 * C:3 * C], in_=qw_r[:, :, 2 * C:3 * C])
            pw_sb = consts.tile([128, CC, C], BF16, tag="pw")
            nc.sync.dma_start(out=pw_sb, in_=pw_d.ap().rearrange("(cc p) o -> p cc o", p=128))

            xn_sb = consts.tile([128, CC, HW], BF16, tag="xn")
            qk_sb = consts.tile([128, OCQK, HW], BF16, tag="qk")
            vt8 = consts.tile([128, 4, 2, NHEADS, HDP], FP8, tag="vt")
            att_t = [consts.tile([128, HW], BF16, tag=f"att{i}", name=f"att{i}") for i in range(CC)]

            # ones column of vT (softmax denominator trick)
            nc.vector.memset(vt8[:, :, :, :, HD:HD + 1], 1.0)
            # exp bias constant (E scaled by 1/16 for e4m3 range)
            ebias = consts.tile([128, 1], F32, tag="ebias")
            nc.vector.memset(ebias, EBIAS)

            # ---- GroupNorm, per-chunk so xn[cc] unblocks as x[cc] lands ----
            for cc in range(CC):
                st = small.tile([128, 2, 6], F32, tag="gn_st", name=f"gn_st{cc}")
                nc.vector.bn_stats(out=st[:, 0, :], in_=x_sb[:, cc, 0:512])
                nc.vector.bn_stats(out=st[:, 1, :], in_=x_sb[:, cc, 512:1024])
                mv = small.tile([128, 2], F32, tag="gn_mv", name=f"gn_mv{cc}")
                nc.vector.bn_aggr(out=mv, in_=st)
                scr = small.tile([128, 1], F32, tag="gn_scr", name=f"gn_scr{cc}")
                nc.vector.tensor_mul(out=scr, in0=mv[:, 0:1], in1=mv[:, 0:1])
                nc.vector.tensor_add(out=mv[:, 1:2], in0=mv[:, 1:2], in1=scr)
                pg = ps_av.tile([8, 2], F32, tag="avB", bufs=1, name=f"gn_pg{cc}")
                nc.tensor.matmul(out=pg, lhsT=ind, rhs=mv, start=True, stop=True)
                # sg cols: [mean_g, ex2->rstd_g, vpe]
                sg = small.tile([8, 4], F32, tag="gn_sg", name=f"gn_sg{cc}")
                nc.vector.tensor_copy(out=sg[:, 0:2], in_=pg)
                nc.vector.scalar_tensor_tensor(out=sg[:, 2:3], in0=sg[:, 0:1], scalar=-1.0, in1=sg[:, 0:1], op0=ALU.mult, op1=ALU.mult)
                nc.vector.scalar_tensor_tensor(out=sg[:, 2:3], in0=sg[:, 1:2], scalar=EPS, in1=sg[:, 2:3], op0=ALU.add, op1=ALU.add)
                nc.scalar.activation(out=sg[:, 3:4], in_=sg[:, 2:3], func=AF.Sqrt, bias=0.0, scale=1.0)
                nc.vector.reciprocal(out=sg[:, 1:2], in_=sg[:, 3:4])
                pbc = ps_av.tile([128, 2], F32, tag="avB", bufs=1, name=f"gn_pbc{cc}")
                nc.tensor.matmul(out=pbc, lhsT=indt, rhs=sg[:, 0:2], start=True, stop=True)
                ab = small.tile([128, 2], F32, tag="gn_ab", name=f"gn_ab{cc}")
                nc.vector.tensor_mul(out=ab[:, 0:1], in0=pbc[:, 1:2], in1=gnw[:, cc:cc + 1])
                nc.vector.scalar_tensor_tensor(out=ab[:, 1:2], in0=pbc[:, 0:1], scalar=-1.0, in1=ab[:, 0:1], op0=ALU.mult, op1=ALU.mult)
                nc.vector.tensor_add(out=ab[:, 1:2], in0=gnb[:, cc:cc + 1], in1=ab[:, 1:2])
                nc.vector.tensor_scalar(out=xn_sb[:, cc, :], in0=x_sb[:, cc, :], scalar1=ab[:, 0:1], scalar2=ab[:, 1:2], op0=ALU.mult, op1=ALU.add)

            # ---- q/k production ----
            def make_qk(oc, pool, tag, bias_on_scalar=False):
                pq = pool.tile([128, HW], F32, tag=tag, bufs=None if tag == "ps_s" else 1, name=f"pq{oc}")
                for cc in range(CC):
                    for q2 in range(QC):
                        nc.tensor.matmul(
                            out=pq[:, q2 * 512:(q2 + 1) * 512],
                            lhsT=qw_sb[:, cc, oc * 128:(oc + 1) * 128],
                            rhs=xn_sb[:, cc, q2 * 512:(q2 + 1) * 512],
                            start=(cc == 0), stop=(cc == CC - 1),
                        )
                if bias_on_scalar:
                    nc.scalar.add(out=qk_sb[:, oc, :], in_=pq[:], add=qkb[:, oc:oc + 1])
                else:
                    nc.vector.tensor_scalar_add(out=qk_sb[:, oc, :], in0=pq[:], scalar1=qkb[:, oc:oc + 1])

            make_qk(0, ps_s, "ps_s", bias_on_scalar=True)
            make_qk(4, ps_s, "ps_s", bias_on_scalar=True)

            # ---- V^T chunks (2 hw-chunks per psum tile), woven into pair 0 ----
            def vt_chunk(i):
                pv = ps_av.tile([128, 2, 512], F32, tag="avB", bufs=1, name=f"pv{i}")
                for h2 in range(2):
                    hwc = 2 * i + h2
                    for cc in range(CC):
                        nc.tensor.matmul(
                            out=pv[:, h2, :],
                            lhsT=xn_sb[:, cc, hwc * 128:(hwc + 1) * 128],
                            rhs=qw_sb[:, cc, 2 * C:3 * C],
                            start=(cc == 0), stop=(cc == CC - 1),
                        )
                for h2 in range(2):
                    hwc = 2 * i + h2
                    nc.vector.tensor_add(
                        out=vt8[:, hwc // 2, hwc % 2, :, 0:HD],
                        in0=pv[:, h2, :].rearrange("p (h d) -> p h d", d=HD),
                        in1=vbb[:].rearrange("p (h d) -> p h d", d=HD),
                    )

            # ---- attention ----
            def normalize_head(hp, a, av_tile):
                avs = small.tile([65, HW], F32, tag="avs", name=f"avs{hp}_{a}")
                nc.vector.tensor_copy(out=avs, in_=av_tile[:, :])
                # denominator row -> DRAM -> [128, 8] so the reciprocal runs
                # across all partitions (recip on a [1, N] row is 6.5us / broken
                # in approx form), then back out to DRAM for the broadcast
                dd = drp.tile([HW], F32, tag="dd", name=f"dd{hp}_{a}")
                nc.sync.dma_start(out=dd, in_=avs[64:65, :])
                dt = small.tile([128, 8], F32, tag="dt", name=f"dt{hp}_{a}")
                nc.sync.dma_start(
                    out=dt,
                    in_=bass.AP(tensor=dd.tensor, offset=dd.offset, ap=[[8, 128], [1, 8]]),
                )
                nc.vector.reciprocal(out=dt, in_=dt)
                rr = drp.tile([HW], F32, tag="rr", name=f"rr{hp}_{a}")
                nc.sync.dma_start(out=rr, in_=dt)
                sbc = small.tile([64, HW], F32, tag="sbc", name=f"sbc{hp}_{a}")
                nc.sync.dma_start(
                    out=sbc,
                    in_=bass.AP(tensor=rr.tensor, offset=rr.offset, ap=[[0, 64]] + list(rr.ap)),
                )
                # partition-shifted DVE write places the odd head at 64:128
                nc.vector.tensor_mul(out=att_t[hp][a * 64:(a + 1) * 64, :], in0=avs[0:64, :], in1=sbc)

            def av_dr(av_tile, head, E_tile, a, kcp, start, stop):
                # fp8 DoubleRow: contracts k-chunks 2*kcp and 2*kcp+1 in one pass
                for q2 in range(QC):
                    w = slice(q2 * 512, (q2 + 1) * 512)
                    nc.tensor.matmul(
                        out=av_tile[:, w],
                        lhsT=vt8[:, kcp, :, head, 0:HD + 1],
                        rhs=E_tile[:, a, 2 * kcp:2 * kcp + 2, w],
                        start=start, stop=stop, perf_mode=DR,
                    )

            E_prev = None
            av0_prev = None

            for hp in range(4):
                E = epool.tile([128, 2, 8, HW], FP8, tag="E", name=f"E{hp}")
                av1 = ps_av.tile([65, HW], F32, tag="avA", bufs=1, name=f"av1_{hp}")
                av0_cur = None
                if hp == 3:
                    av0_cur = ps_av.tile([65, HW], F32, tag="avB", bufs=1, name="av0_3")
                for kc in range(8):
                    if hp == 0 and kc % 2 == 0:
                        vt_chunk(kc // 2)
                    psA = ps_s.tile([128, HW], F32, tag="ps_s", name=f"s{hp}_{kc}_1")
                    psB = ps_s.tile([128, HW], F32, tag="ps_s", name=f"s{hp}_{kc}_0")
                    for q2 in range(QC):
                        w = slice(q2 * 512, (q2 + 1) * 512)
                        nc.tensor.matmul(
                            out=psA[:, w],
                            lhsT=qk_sb[64:128, 4 + hp, kc * 128:(kc + 1) * 128],
                            rhs=qk_sb[64:128, hp, w],
                            start=True, stop=True,
                        )
                        nc.tensor.matmul(
                            out=psB[:, w],
                            lhsT=qk_sb[0:64, 4 + hp, kc * 128:(kc + 1) * 128],
                            rhs=qk_sb[0:64, hp, w],
                            start=True, stop=True,
                        )
                    nc.scalar.activation(out=E[:, 1, kc, :], in_=psA[:], func=AF.Exp, scale=SCALE, bias=ebias[:, 0:1])
                    nc.scalar.activation(out=E[:, 0, kc, :], in_=psB[:], func=AF.Exp, scale=SCALE, bias=ebias[:, 0:1])
                    # head a=1 AV: one DoubleRow pass per completed kc pair
                    if kc % 2 == 1:
                        av_dr(av1, 2 * hp + 1, E, 1, kc // 2, start=(kc == 1), stop=(kc == 7))
                    if hp < 3:
                        # previous pair's a=0 AV spread over kc 1..4
                        if E_prev is not None and 1 <= kc <= 4:
                            av_dr(av0_prev, 2 * (hp - 1), E_prev, 0, kc - 1,
                                  start=(kc == 1), stop=(kc == 4))
                            if kc == 4:
                                normalize_head(hp - 1, 0, av0_prev)
                        if kc == 4:
                            make_qk(hp + 1, ps_av, "avB")
                        if kc == 6:
                            make_qk(4 + hp + 1, ps_av, "avB")
                    else:
                        # last pair: finish pair 2's a=0 early, stream own a=0
                        if kc in (0, 1):
                            for bkcp in (2 * kc, 2 * kc + 1):
                                av_dr(av0_prev, 4, E_prev, 0, bkcp,
                                      start=(bkcp == 0), stop=(bkcp == 3))
                            if kc == 1:
                                normalize_head(2, 0, av0_prev)
                        if kc in (5, 7):
                            av_dr(av0_cur, 6, E, 0, kc // 2, start=(kc == 5), stop=False)
                if hp == 3:
                    for bkcp in (0, 1):
                        av_dr(av0_cur, 6, E, 0, bkcp, start=False, stop=(bkcp == 1))
                normalize_head(hp, 1, av1)
                if hp == 3:
                    normalize_head(3, 0, av0_cur)
                E_prev = E
                if hp < 3:
                    av0_prev = ps_av.tile([65, HW], F32, tag="avB", bufs=1, name=f"av0_{hp}")

            # ---- proj + residual tail ----
            def proj_mm(py, oc, cc):
                for q2 in range(QC):
                    nc.tensor.matmul(
                        out=py[:, q2 * 512:(q2 + 1) * 512],
                        lhsT=pw_sb[:, cc, oc * 128:(oc + 1) * 128],
                        rhs=att_t[cc][:, q2 * 512:(q2 + 1) * 512],
                        start=(cc == 0), stop=(cc == CC - 1),
                    )

            def proj_epilogue(py, oc):
                ot = outp.tile([128, HW], F32, tag="ot", name=f"ot{oc}")
                nc.vector.scalar_tensor_tensor(out=ot, in0=py[:], scalar=pb[:, oc:oc + 1], in1=x_sb[:, oc, :], op0=ALU.add, op1=ALU.add)
                nc.sync.dma_start(out=out_d.ap()[oc * 128:(oc + 1) * 128, :], in_=ot)

            pys = []
            for oc, (pool, tag) in enumerate(
                ((ps_s, "ps_s"), (ps_s, "ps_s"), (ps_av, "avA"), (ps_av, "avB"))
            ):
                py = pool.tile([128, HW], F32, tag=tag, bufs=None if tag == "ps_s" else 1, name=f"py{oc}")
                pys.append(py)
                for cc in range(CC - 1):
                    proj_mm(py, oc, cc)
            for oc in range(4):
                proj_mm(pys[oc], oc, CC - 1)
                proj_epilogue(pys[oc], oc)

    nc.compile()
    return nc


_NC_CACHE = None


def _get_nc():
    global _NC_CACHE
    if _NC_CACHE is None:
        _NC_CACHE = _build()
    return _NC_CACHE


def _prep_in_maps(inputs):
    x = np.asarray(inputs["x"], np.float32)
    gn_w = np.asarray(inputs["gn_w"], np.float32)
    gn_b = np.asarray(inputs["gn_b"], np.float32)
    qkv_w = np.asarray(inputs["qkv_w"], np.float32)
    qkv_b = np.asarray(inputs["qkv_b"], np.float32)
    proj_w = np.asarray(inputs["proj_w"], np.float32)
    proj_b = np.asarray(inputs["proj_b"], np.float32)

    B = x.shape[0]
    xr = x.reshape(B, C, HW).astype(ml_dtypes.bfloat16)
    qwT = np.ascontiguousarray(qkv_w.T).astype(ml_dtypes.bfloat16)
    pwT = np.ascontiguousarray(proj_w.T).astype(ml_dtypes.bfloat16)
    qkb = np.ascontiguousarray(qkv_b[: 2 * C].reshape(8, 128).T)
    vbb = np.ascontiguousarray(np.broadcast_to(qkv_b[2 * C:], (128, C)))
    pb = np.ascontiguousarray(proj_b.reshape(4, 128).T)
    gnw = np.ascontiguousarray(gn_w.reshape(4, 128).T)
    gnb = np.ascontiguousarray(gn_b.reshape(4, 128).T)
    indm = np.zeros((128, 8), np.float32)
    indm[np.arange(128), np.arange(128) // GSIZE] = 1.0 / GSIZE
    ind01 = np.zeros((128, 8), np.float32)
    ind01[np.arange(128), np.arange(128) // GSIZE] = 1.0
    indt = np.ascontiguousarray(ind01.T)
    shared = dict(qw=qwT, pw=pwT, qkb=qkb, vbb=vbb, pb=pb, gnw=gnw, gnb=gnb, ind=indm, indt=indt)
    return [dict(x=np.ascontiguousarray(xr[b]), **shared) for b in range(B)]


def kernel(**inputs):
    nc = _get_nc()
    in_maps = _prep_in_maps(inputs)
    res = run_bass_kernel_spmd(nc, in_maps, core_ids=list(range(8)))
    out = np.stack([r["out"] for r in res.results])
    return out.reshape(8, C, 32, 32).astype(np.float32)


def run_profiled(inputs):
    """kernel() + NTFF profiling; returns (output, exec_time_ns, trace_path)."""
    import types

    import antenv

    if "antenv.axon_hooks" not in sys.modules:
        hooks_mod = types.ModuleType("antenv.axon_hooks")
        _hook = [None]
        hooks_mod.set_axon_ntff_profile_hook = lambda h: _hook.__setitem__(0, h)
        hooks_mod.get_axon_ntff_profile_hook = lambda: _hook[0]
        sys.modules["antenv.axon_hooks"] = hooks_mod
        antenv.axon_hooks = hooks_mod
        from trn_agent_boot.trn_boot import _ntff_profile_via_ctypes

        hooks_mod.set_axon_ntff_profile_hook(_ntff_profile_via_ctypes("/opt/axon/libaxon_pjrt.so"))

    nc = _get_nc()
    in_maps = _prep_in_maps(inputs)
    res = run_bass_kernel_spmd(nc, in_maps, core_ids=list(range(8)), trace=True)
    out = np.stack([r["out"] for r in res.results]).reshape(8, C, 32, 32).astype(np.float32)
    trace = res.instructions_and_trace[1] if res.instructions_and_trace else None
    return out, res.exec_time_ns, trace


# revision 9
# speedup vs baseline: 1.2255x; 1.2255x over previous
"""AttentionBlock Trainium2 kernel: GroupNorm -> QKV -> MHA -> proj -> residual.

Data-parallel over batch B=8 across 8 NeuronCores (one batch image per core).
QKV/proj/score matmuls run in bf16 (fp32 PSUM accumulation); the
attention-value matmuls run in fp8 e4m3 DoubleRow (2 k-chunks per pass):
E is stored as exp(s*scale)/16 in e4m3 (the /16 keeps e4m3 in range; it
rides both the numerator and the ones-column denominator so it cancels).

Per-core layouts (C=512 channels, HW=1024 tokens, 8 heads, hd=64):
  x            [C, HW] bf16 (stats + residual tolerate bf16)
  xn           [C, HW] bf16, per-128-channel-chunk GroupNorm
  q, k         qk_sb[128, 8 oc, 1024] bf16; pair hp: oc=hp (q), 4+hp (k),
               head a at partitions a*64:(a+1)*64
  vT (fp8)     vt8[128, kcp 4, ko 2, head 8, 68] (65 used: 64 + ones col)
  scoresT      [k, q] PSUM [128, 1024] per (head, kc)
  E            [128, 2 head, 8 kc, 1024] fp8 per pair
  att          [C, HW] bf16; out [C, HW] fp32 = proj(att) + pb + x

PSUM (8 banks): ps_s x2 (scores / lead-in qk / proj py0-1) = 4 banks;
avA x1 (in-pair AV accumulator / py2) + avB x1 (rotating: GN stats, V^T,
mid-pair qk, deferred a=0 AV, py3) = 4 banks.

Softmax denominators: AV psum row 64 -> SBUF; reciprocal_approx_fast on the
row; DRAM bounce broadcast to [64, 1024]; multiply (partition-shifted write
for the odd head).
"""

import sys

if "/opt/trn_rl_repo" not in sys.path:
    sys.path.insert(0, "/opt/trn_rl_repo")

import numpy as np
import ml_dtypes

import concourse.bass as bass
import concourse.tile as tile
from concourse import mybir, bacc
from concourse.bass_utils import run_bass_kernel_spmd

AF = mybir.ActivationFunctionType
ALU = mybir.AluOpType
F32 = mybir.dt.float32
BF16 = mybir.dt.bfloat16
FP8 = mybir.dt.float8e4
DR = mybir.MatmulPerfMode.DoubleRow

C = 512
HW = 1024
NHEADS = 8
HD = 64
HDP = 68  # padded head stride so the DoubleRow ko-step (8*HDP) is 16-aligned
NGROUPS = 32
GSIZE = 16
EPS = 1e-5
SCALE = HD ** -0.5
EBIAS = -4.0 * float(np.log(2.0))  # exp output scaled by 1/16 for e4m3 range
CC = 4
OCQK = 8
QC = 2


def _build():
    nc = bacc.Bacc("TRN2", target_bir_lowering=False, debug=False, num_devices=8)

    x_d = nc.dram_tensor("x", [C, HW], BF16, kind="ExternalInput")
    qw_d = nc.dram_tensor("qw", [C, 3 * C], BF16, kind="ExternalInput")
    pw_d = nc.dram_tensor("pw", [C, C], BF16, kind="ExternalInput")
    qkb_d = nc.dram_tensor("qkb", [128, 8], F32, kind="ExternalInput")
    vbb_d = nc.dram_tensor("vbb", [128, C], F32, kind="ExternalInput")
    pb_d = nc.dram_tensor("pb", [128, 4], F32, kind="ExternalInput")
    gnw_d = nc.dram_tensor("gnw", [128, 4], F32, kind="ExternalInput")
    gnb_d = nc.dram_tensor("gnb", [128, 4], F32, kind="ExternalInput")
    ind_d = nc.dram_tensor("ind", [128, 8], F32, kind="ExternalInput")
    indt_d = nc.dram_tensor("indt", [8, 128], F32, kind="ExternalInput")
    out_d = nc.dram_tensor("out", [C, HW], F32, kind="ExternalOutput")

    with tile.TileContext(nc) as tc:
        with (
            tc.tile_pool(name="consts", bufs=1) as consts,
            tc.tile_pool(name="epool", bufs=2) as epool,
            tc.tile_pool(name="small", bufs=4) as small,
            tc.tile_pool(name="outp", bufs=3) as outp,
            tc.tile_pool(name="drp", bufs=4, space="DRAM") as drp,
            tc.tile_pool(name="ps_s", bufs=2, space="PSUM") as ps_s,
            tc.tile_pool(name="ps_av", bufs=1, space="PSUM") as ps_av,
        ):
            # ---- input DMAs: x chunks first (GN gates on them), then tables ----
            x_sb = consts.tile([128, CC, HW], BF16, tag="x")
            x_r = x_d.ap().rearrange("(cc p) hw -> p cc hw", p=128)
            for cc in range(CC):
                nc.sync.dma_start(out=x_sb[:, cc, :], in_=x_r[:, cc, :])
            ind = consts.tile([128, 8], F32, tag="ind")
            nc.gpsimd.dma_start(out=ind, in_=ind_d.ap())
            indt = consts.tile([8, 128], F32, tag="indt")
            nc.gpsimd.dma_start(out=indt, in_=indt_d.ap())
            gnw = consts.tile([128, 4], F32, tag="gnw")
            nc.gpsimd.dma_start(out=gnw, in_=gnw_d.ap())
            gnb = consts.tile([128, 4], F32, tag="gnb")
            nc.gpsimd.dma_start(out=gnb, in_=gnb_d.ap())
            qkb = consts.tile([128, 8], F32, tag="qkb")
            nc.gpsimd.dma_start(out=qkb, in_=qkb_d.ap())
            vbb = consts.tile([128, C], F32, tag="vbb")
            nc.gpsimd.dma_start(out=vbb, in_=vbb_d.ap())
            pb = consts.tile([128, 4], F32, tag="pb")
            nc.gpsimd.dma_start(out=pb, in_=pb_d.ap())
            qw_sb = consts.tile([128, CC, 3 * C], BF16, tag="qw")
            qw_r = qw_d.ap().rearrange("(cc p) o -> p cc o", p=128)
            nc.sync.dma_start(out=qw_sb[:, :, 0:2 * C], in_=qw_r[:, :, 0:2 * C])
            nc.sync.dma_start(out=qw_sb[:, :, 2 * C:3 * C], in_=qw_r[:, :, 2 * C:3 * C])
            pw_sb = consts.tile([128, CC, C], BF16, tag="pw")
            nc.sync.dma_start(out=pw_sb, in_=pw_d.ap().rearrange("(cc p) o -> p cc o", p=128))

            xn_sb = consts.tile([128, CC, HW], BF16, tag="xn")
            qk_sb = consts.tile([128, OCQK, HW], BF16, tag="qk")
            vt8 = consts.tile([128, 4, 2, NHEADS, HDP], FP8, tag="vt")
            att_t = [consts.tile([128, HW], BF16, tag=f"att{i}", name=f"att{i}") for i in range(CC)]

            # ones column of vT (softmax denominator trick)
            nc.vector.memset(vt8[:, :, :, :, HD:HD + 1], 1.0)
            # exp bias constant (E scaled by 1/16 for e4m3 range)
            ebias = consts.tile([128, 1], F32, tag="ebias")
            nc.vector.memset(ebias, EBIAS)

            # ---- GroupNorm, per-chunk so xn[cc] unblocks as x[cc] lands ----
            for cc in range(CC):
                st = small.tile([128, 2, 6], F32, tag="gn_st", name=f"gn_st{cc}")
                nc.vector.bn_stats(out=st[:, 0, :], in_=x_sb[:, cc, 0:512])
                nc.vector.bn_stats(out=st[:, 1, :], in_=x_sb[:, cc, 512:1024])
                mv = small.tile([128, 2], F32, tag="gn_mv", name=f"gn_mv{cc}")
                nc.vector.bn_aggr(out=mv, in_=st)
                scr = small.tile([128, 1], F32, tag="gn_scr", name=f"gn_scr{cc}")
                nc.vector.tensor_mul(out=scr, in0=mv[:, 0:1], in1=mv[:, 0:1])
                nc.vector.tensor_add(out=mv[:, 1:2], in0=mv[:, 1:2], in1=scr)
                pg = ps_av.tile([8, 2], F32, tag="avB", bufs=1, name=f"gn_pg{cc}")
                nc.tensor.matmul(out=pg, lhsT=ind, rhs=mv, start=True, stop=True)
                # sg cols: [mean_g, ex2->rstd_g, vpe]
                sg = small.tile([8, 4], F32, tag="gn_sg", name=f"gn_sg{cc}")
                nc.vector.tensor_copy(out=sg[:, 0:2], in_=pg)
                nc.vector.scalar_tensor_tensor(out=sg[:, 2:3], in0=sg[:, 0:1], scalar=-1.0, in1=sg[:, 0:1], op0=ALU.mult, op1=ALU.mult)
                nc.vector.scalar_tensor_tensor(out=sg[:, 2:3], in0=sg[:, 1:2], scalar=EPS, in1=sg[:, 2:3], op0=ALU.add, op1=ALU.add)
                nc.scalar.activation(out=sg[:, 3:4], in_=sg[:, 2:3], func=AF.Sqrt, bias=0.0, scale=1.0)
                nc.vector.reciprocal(out=sg[:, 1:2], in_=sg[:, 3:4])
                pbc = ps_av.tile([128, 2], F32, tag="avB", bufs=1, name=f"gn_pbc{cc}")
                nc.tensor.matmul(out=pbc, lhsT=indt, rhs=sg[:, 0:2], start=True, stop=True)
                ab = small.tile([128, 2], F32, tag="gn_ab", name=f"gn_ab{cc}")
                nc.vector.tensor_mul(out=ab[:, 0:1], in0=pbc[:, 1:2], in1=gnw[:, cc:cc + 1])
                nc.vector.scalar_tensor_tensor(out=ab[:, 1:2], in0=pbc[:, 0:1], scalar=-1.0, in1=ab[:, 0:1], op0=ALU.mult, op1=ALU.mult)
                nc.vector.tensor_add(out=ab[:, 1:2], in0=gnb[:, cc:cc + 1], in1=ab[:, 1:2])
                nc.vector.tensor_scalar(out=xn_sb[:, cc, :], in0=x_sb[:, cc, :], scalar1=ab[:, 0:1], scalar2=ab[:, 1:2], op0=ALU.mult, op1=ALU.add)

            # ---- q/k production ----
            def make_qk(oc, pool, tag, bias_on_scalar=False):
                pq = pool.tile([128, HW], F32, tag=tag, bufs=None if tag == "ps_s" else 1, name=f"pq{oc}")
                for cc in range(CC):
                    for q2 in range(QC):
                        nc.tensor.matmul(
                            out=pq[:, q2 * 512:(q2 + 1) * 512],
                            lhsT=qw_sb[:, cc, oc * 128:(oc + 1) * 128],
                            rhs=xn_sb[:, cc, q2 * 512:(q2 + 1) * 512],
                            start=(cc == 0), stop=(cc == CC - 1),
                        )
                if bias_on_scalar:
                    nc.scalar.add(out=qk_sb[:, oc, :], in_=pq[:], add=qkb[:, oc:oc + 1])
                else:
                    nc.vector.tensor_scalar_add(out=qk_sb[:, oc, :], in0=pq[:], scalar1=qkb[:, oc:oc + 1])

            make_qk(0, ps_s, "ps_s", bias_on_scalar=True)
            make_qk(4, ps_s, "ps_s", bias_on_scalar=True)

            # ---- V^T chunks (2 hw-chunks per psum tile), woven into pair 0 ----
            def vt_chunk(i):
                pv = ps_av.tile([128, 2, 512], F32, tag="avB", bufs=1, name=f"pv{i}")
                for h2 in range(2):
                    hwc = 2 * i + h2
                    for cc in range(CC):
                        nc.tensor.matmul(
                            out=pv[:, h2, :],
                            lhsT=xn_sb[:, cc, hwc * 128:(hwc + 1) * 128],
                            rhs=qw_sb[:, cc, 2 * C:3 * C],
                            start=(cc == 0), stop=(cc == CC - 1),
                        )
                for h2 in range(2):
                    hwc = 2 * i + h2
                    nc.vector.tensor_add(
                        out=vt8[:, hwc // 2, hwc % 2, :, 0:HD],
                        in0=pv[:, h2, :].rearrange("p (h d) -> p h d", d=HD),
                        in1=vbb[:].rearrange("p (h d) -> p h d", d=HD),
                    )

            # ---- attention ----
            def normalize_head(hp, a, av_tile):
                avs = small.tile([65, HW], F32, tag="avs", name=f"avs{hp}_{a}")
                nc.vector.tensor_copy(out=avs, in_=av_tile[:, :])
                # denominator row -> DRAM -> [128, 8] so the reciprocal runs
                # across all partitions (recip on a [1, N] row is 6.5us / broken
                # in approx form), then back out to DRAM for the broadcast
                dd = drp.tile([HW], F32, tag="dd", name=f"dd{hp}_{a}")
                nc.gpsimd.dma_start(out=dd, in_=avs[64:65, :])
                dt = small.tile([128, 8], F32, tag="dt", name=f"dt{hp}_{a}")
                nc.gpsimd.dma_start(
                    out=dt,
                    in_=bass.AP(tensor=dd.tensor, offset=dd.offset, ap=[[8, 128], [1, 8]]),
                )
                nc.vector.reciprocal(out=dt, in_=dt)
                rr = drp.tile([HW], F32, tag="rr", name=f"rr{hp}_{a}")
                nc.gpsimd.dma_start(out=rr, in_=dt)
                sbc = small.tile([64, HW], F32, tag="sbc", name=f"sbc{hp}_{a}")
                nc.gpsimd.dma_start(
                    out=sbc,
                    in_=bass.AP(tensor=rr.tensor, offset=rr.offset, ap=[[0, 64]] + list(rr.ap)),
                )
                # partition-shifted DVE write places the odd head at 64:128
                nc.vector.tensor_mul(out=att_t[hp][a * 64:(a + 1) * 64, :], in0=avs[0:64, :], in1=sbc)

            def av_dr(av_tile, head, E_tile, a, kcp, start, stop):
                # fp8 DoubleRow: contracts k-chunks 2*kcp and 2*kcp+1 in one pass
                for q2 in range(QC):
                    w = slice(q2 * 512, (q2 + 1) * 512)
                    nc.tensor.matmul(
                        out=av_tile[:, w],
                        lhsT=vt8[:, kcp, :, head, 0:HD + 1],
                        rhs=E_tile[:, a, 2 * kcp:2 * kcp + 2, w],
                        start=start, stop=stop, perf_mode=DR,
                    )

            E_prev = None
            av0_prev = None

            for hp in range(4):
                E = epool.tile([128, 2, 8, HW], FP8, tag="E", name=f"E{hp}")
                av1 = ps_av.tile([65, HW], F32, tag="avA", bufs=1, name=f"av1_{hp}")
                av0_cur = None
                if hp == 3:
                    av0_cur = ps_av.tile([65, HW], F32, tag="avB", bufs=1, name="av0_3")
                for kc in range(8):
                    if hp == 0 and kc % 2 == 0:
                        vt_chunk(kc // 2)
                    psA = ps_s.tile([128, HW], F32, tag="ps_s", name=f"s{hp}_{kc}_1")
                    psB = ps_s.tile([128, HW], F32, tag="ps_s", name=f"s{hp}_{kc}_0")
                    for q2 in range(QC):
                        w = slice(q2 * 512, (q2 + 1) * 512)
                        nc.tensor.matmul(
                            out=psA[:, w],
                            lhsT=qk_sb[64:128, 4 + hp, kc * 128:(kc + 1) * 128],
                            rhs=qk_sb[64:128, hp, w],
                            start=True, stop=True,
                        )
                        nc.tensor.matmul(
                            out=psB[:, w],
                            lhsT=qk_sb[0:64, 4 + hp, kc * 128:(kc + 1) * 128],
                            rhs=qk_sb[0:64, hp, w],
                            start=True, stop=True,
                        )
                    nc.scalar.activation(out=E[:, 1, kc, :], in_=psA[:], func=AF.Exp, scale=SCALE, bias=ebias[:, 0:1])
                    nc.scalar.activation(out=E[:, 0, kc, :], in_=psB[:], func=AF.Exp, scale=SCALE, bias=ebias[:, 0:1])
                    # head a=1 AV: one DoubleRow pass per completed kc pair
                    if kc % 2 == 1:
                        av_dr(av1, 2 * hp + 1, E, 1, kc // 2, start=(kc == 1), stop=(kc == 7))
                    if hp < 3:
                        # previous pair's a=0 AV spread over kc 1..4
                        if E_prev is not None and 1 <= kc <= 4:
                            av_dr(av0_prev, 2 * (hp - 1), E_prev, 0, kc - 1,
                                  start=(kc == 1), stop=(kc == 4))
                            if kc == 4:
                                normalize_head(hp - 1, 0, av0_prev)
                        if kc == 4:
                            make_qk(hp + 1, ps_av, "avB")
                        if kc == 6:
                            make_qk(4 + hp + 1, ps_av, "avB")
                    else:
                        # last pair: finish pair 2's a=0 early, stream own a=0
                        if kc in (0, 1):
                            for bkcp in (2 * kc, 2 * kc + 1):
                                av_dr(av0_prev, 4, E_prev, 0, bkcp,
                                      start=(bkcp == 0), stop=(bkcp == 3))
                            if kc == 1:
                                normalize_head(2, 0, av0_prev)
                        if kc in (5, 7):
                            av_dr(av0_cur, 6, E, 0, kc // 2, start=(kc == 5), stop=False)
                if hp == 3:
                    for bkcp in (0, 1):
                        av_dr(av0_cur, 6, E, 0, bkcp, start=False, stop=(bkcp == 1))
                normalize_head(hp, 1, av1)
                if hp == 3:
                    normalize_head(3, 0, av0_cur)
                E_prev = E
                if hp < 3:
                    av0_prev = ps_av.tile([65, HW], F32, tag="avB", bufs=1, name=f"av0_{hp}")

            # ---- proj + residual tail ----
            def proj_mm(py, oc, cc):
                for q2 in range(QC):
                    nc.tensor.matmul(
                        out=py[:, q2 * 512:(q2 + 1) * 512],
                        lhsT=pw_sb[:, cc, oc * 128:(oc + 1) * 128],
                        rhs=att_t[cc][:, q2 * 512:(q2 + 1) * 512],
                        start=(cc == 0), stop=(cc == CC - 1),
                    )

            def proj_epilogue(py, oc):
                ot = outp.tile([128, HW], F32, tag="ot", name=f"ot{oc}")
                nc.vector.scalar_tensor_tensor(out=ot, in0=py[:], scalar=pb[:, oc:oc + 1], in1=x_sb[:, oc, :], op0=ALU.add, op1=ALU.add)
                nc.sync.dma_start(out=out_d.ap()[oc * 128:(oc + 1) * 128, :], in_=ot)

            pys = []
            for oc, (pool, tag) in enumerate(
                ((ps_s, "ps_s"), (ps_s, "ps_s"), (ps_av, "avA"), (ps_av, "avB"))
            ):
                py = pool.tile([128, HW], F32, tag=tag, bufs=None if tag == "ps_s" else 1, name=f"py{oc}")
                pys.append(py)
                for cc in range(CC - 1):
                    proj_mm(py, oc, cc)
            for oc in range(4):
                proj_mm(pys[oc], oc, CC - 1)
                proj_epilogue(pys[oc], oc)

    nc.compile()
    return nc


_NC_CACHE = None


def _get_nc():
    global _NC_CACHE
    if _NC_CACHE is None:
        _NC_CACHE = _build()
    return _NC_CACHE


def _prep_in_maps(inputs):
    x = np.asarray(inputs["x"], np.float32)
    gn_w = np.asarray(inputs["gn_w"], np.float32)
    gn_b = np.asarray(inputs["gn_b"], np.float32)
    qkv_w = np.asarray(inputs["qkv_w"], np.float32)
    qkv_b = np.asarray(inputs["qkv_b"], np.float32)
    proj_w = np.asarray(inputs["proj_w"], np.float32)
    proj_b = np.asarray(inputs["proj_b"], np.float32)

    B = x.shape[0]
    xr = x.reshape(B, C, HW).astype(ml_dtypes.bfloat16)
    qwT = np.ascontiguousarray(qkv_w.T).astype(ml_dtypes.bfloat16)
    pwT = np.ascontiguousarray(proj_w.T).astype(ml_dtypes.bfloat16)
    qkb = np.ascontiguousarray(qkv_b[: 2 * C].reshape(8, 128).T)
    vbb = np.ascontiguousarray(np.broadcast_to(qkv_b[2 * C:], (128, C)))
    pb = np.ascontiguousarray(proj_b.reshape(4, 128).T)
    gnw = np.ascontiguousarray(gn_w.reshape(4, 128).T)
    gnb = np.ascontiguousarray(gn_b.reshape(4, 128).T)
    indm = np.zeros((128, 8), np.float32)
    indm[np.arange(128), np.arange(128) // GSIZE] = 1.0 / GSIZE
    ind01 = np.zeros((128, 8), np.float32)
    ind01[np.arange(128), np.arange(128) // GSIZE] = 1.0
    indt = np.ascontiguousarray(ind01.T)
    shared = dict(qw=qwT, pw=pwT, qkb=qkb, vbb=vbb, pb=pb, gnw=gnw, gnb=gnb, ind=indm, indt=indt)
    return [dict(x=np.ascontiguousarray(xr[b]), **shared) for b in range(B)]


def kernel(**inputs):
    nc = _get_nc()
    in_maps = _prep_in_maps(inputs)
    res = run_bass_kernel_spmd(nc, in_maps, core_ids=list(range(8)))
    out = np.stack([r["out"] for r in res.results])
    return out.reshape(8, C, 32, 32).astype(np.float32)


def run_profiled(inputs):
    """kernel() + NTFF profiling; returns (output, exec_time_ns, trace_path)."""
    import types

    import antenv

    if "antenv.axon_hooks" not in sys.modules:
        hooks_mod = types.ModuleType("antenv.axon_hooks")
        _hook = [None]
        hooks_mod.set_axon_ntff_profile_hook = lambda h: _hook.__setitem__(0, h)
        hooks_mod.get_axon_ntff_profile_hook = lambda: _hook[0]
        sys.modules["antenv.axon_hooks"] = hooks_mod
        antenv.axon_hooks = hooks_mod
        from trn_agent_boot.trn_boot import _ntff_profile_via_ctypes

        hooks_mod.set_axon_ntff_profile_hook(_ntff_profile_via_ctypes("/opt/axon/libaxon_pjrt.so"))

    nc = _get_nc()
    in_maps = _prep_in_maps(inputs)
    res = run_bass_kernel_spmd(nc, in_maps, core_ids=list(range(8)), trace=True)
    out = np.stack([r["out"] for r in res.results]).reshape(8, C, 32, 32).astype(np.float32)
    trace = res.instructions_and_trace[1] if res.instructions_and_trace else None
    return out, res.exec_time_ns, trace


# revision 12
# speedup vs baseline: 1.2669x; 1.0338x over previous
"""AttentionBlock Trainium2 kernel: GroupNorm -> QKV -> MHA -> proj -> residual.

Data-parallel over batch B=8 across 8 NeuronCores (one batch image per core).
QKV/proj/score matmuls run in bf16 (fp32 PSUM accumulation); the
attention-value matmuls run in fp8 e4m3 DoubleRow (2 k-chunks per pass):
E is stored as exp(s*scale)/16 in e4m3 (the /16 keeps e4m3 in range; it
rides both the numerator and the ones-column denominator so it cancels).

Per-core layouts (C=512 channels, HW=1024 tokens, 8 heads, hd=64):
  x            [C, HW] bf16 (stats + residual tolerate bf16)
  xn           [C, HW] bf16, per-128-channel-chunk GroupNorm
  q, k         qk_sb[128, 8 oc, 1024] bf16; pair hp: oc=hp (q), 4+hp (k),
               head a at partitions a*64:(a+1)*64
  vT (fp8)     vt8[128, kcp 4, ko 2, head 8, 68] (65 used: 64 + ones col)
  scoresT      [k, q] PSUM [128, 1024] per (head, kc)
  E            [128, 2 head, 8 kc, 1024] fp8 per pair
  att          [C, HW] bf16; out [C, HW] fp32 = proj(att) + pb + x

PSUM (8 banks): ps_s x2 (scores / lead-in qk / proj py0-1) = 4 banks;
avA x1 (in-pair AV accumulator / py2) + avB x1 (rotating: GN stats, V^T,
mid-pair qk, deferred a=0 AV, py3) = 4 banks.

Softmax denominators: AV psum row 64 -> SBUF; reciprocal_approx_fast on the
row; DRAM bounce broadcast to [64, 1024]; multiply (partition-shifted write
for the odd head).
"""

import sys

if "/opt/trn_rl_repo" not in sys.path:
    sys.path.insert(0, "/opt/trn_rl_repo")

import numpy as np
import ml_dtypes

import concourse.bass as bass
import concourse.tile as tile
from concourse import mybir, bacc
from concourse.bass_utils import run_bass_kernel_spmd

AF = mybir.ActivationFunctionType
ALU = mybir.AluOpType
F32 = mybir.dt.float32
BF16 = mybir.dt.bfloat16
FP8 = mybir.dt.float8e4
DR = mybir.MatmulPerfMode.DoubleRow

C = 512
HW = 1024
NHEADS = 8
HD = 64
HDP = 68  # padded head stride so the DoubleRow ko-step (8*HDP) is 16-aligned
NGROUPS = 32
GSIZE = 16
EPS = 1e-5
SCALE = HD ** -0.5
EBIAS = -4.0 * float(np.log(2.0))  # exp output scaled by 1/16 for e4m3 range
CC = 4
OCQK = 8
QC = 2


def _build():
    nc = bacc.Bacc("TRN2", target_bir_lowering=False, debug=False, num_devices=8)

    x_d = nc.dram_tensor("x", [C, HW], BF16, kind="ExternalInput")
    qw_d = nc.dram_tensor("qw", [C, 3 * C], BF16, kind="ExternalInput")
    pw_d = nc.dram_tensor("pw", [C, C], BF16, kind="ExternalInput")
    qkb_d = nc.dram_tensor("qkb", [128, 8], F32, kind="ExternalInput")
    vbb_d = nc.dram_tensor("vbb", [128, C], F32, kind="ExternalInput")
    pb_d = nc.dram_tensor("pb", [128, 4], F32, kind="ExternalInput")
    gnw_d = nc.dram_tensor("gnw", [128, 4], F32, kind="ExternalInput")
    gnb_d = nc.dram_tensor("gnb", [128, 4], F32, kind="ExternalInput")
    ind_d = nc.dram_tensor("ind", [128, 8], F32, kind="ExternalInput")
    indt_d = nc.dram_tensor("indt", [8, 128], F32, kind="ExternalInput")
    out_d = nc.dram_tensor("out", [C, HW], F32, kind="ExternalOutput")

    with tile.TileContext(nc) as tc:
        with (
            tc.tile_pool(name="consts", bufs=1) as consts,
            tc.tile_pool(name="epool", bufs=2) as epool,
            tc.tile_pool(name="small", bufs=4) as small,
            tc.tile_pool(name="outp", bufs=3) as outp,
            tc.tile_pool(name="drp", bufs=4, space="DRAM") as drp,
            tc.tile_pool(name="ps_s", bufs=2, space="PSUM") as ps_s,
            tc.tile_pool(name="ps_av", bufs=1, space="PSUM") as ps_av,
        ):
            # ---- input DMAs: x chunks first (GN gates on them), then tables ----
            x_sb = consts.tile([128, CC, HW], BF16, tag="x")
            x_r = x_d.ap().rearrange("(cc p) hw -> p cc hw", p=128)
            for cc in range(CC):
                nc.sync.dma_start(out=x_sb[:, cc, :], in_=x_r[:, cc, :])
            ind = consts.tile([128, 8], F32, tag="ind")
            nc.gpsimd.dma_start(out=ind, in_=ind_d.ap())
            indt = consts.tile([8, 128], F32, tag="indt")
            nc.gpsimd.dma_start(out=indt, in_=indt_d.ap())
            gnw = consts.tile([128, 4], F32, tag="gnw")
            nc.gpsimd.dma_start(out=gnw, in_=gnw_d.ap())
            gnb = consts.tile([128, 4], F32, tag="gnb")
            nc.gpsimd.dma_start(out=gnb, in_=gnb_d.ap())
            qkb = consts.tile([128, 8], F32, tag="qkb")
            nc.gpsimd.dma_start(out=qkb, in_=qkb_d.ap())
            vbb = consts.tile([128, C], F32, tag="vbb")
            nc.gpsimd.dma_start(out=vbb, in_=vbb_d.ap())
            pb = consts.tile([128, 4], F32, tag="pb")
            nc.gpsimd.dma_start(out=pb, in_=pb_d.ap())
            qw_sb = consts.tile([128, CC, 3 * C], BF16, tag="qw")
            qw_r = qw_d.ap().rearrange("(cc p) o -> p cc o", p=128)
            nc.sync.dma_start(out=qw_sb[:, :, 0:2 * C], in_=qw_r[:, :, 0:2 * C])
            nc.sync.dma_start(out=qw_sb[:, :, 2 * C:3 * C], in_=qw_r[:, :, 2 * C:3 * C])
            pw_sb = consts.tile([128, CC, C], BF16, tag="pw")
            nc.sync.dma_start(out=pw_sb, in_=pw_d.ap().rearrange("(cc p) o -> p cc o", p=128))

            xn_sb = consts.tile([128, CC, HW], BF16, tag="xn")
            qk_sb = consts.tile([128, OCQK, HW], BF16, tag="qk")
            vt8 = consts.tile([128, 4, 2, NHEADS, HDP], FP8, tag="vt")
            att_t = [consts.tile([128, HW], BF16, tag=f"att{i}", name=f"att{i}") for i in range(CC)]

            # ones column of vT (softmax denominator trick)
            nc.vector.memset(vt8[:, :, :, :, HD:HD + 1], 1.0)
            # exp bias constant (E scaled by 1/16 for e4m3 range)
            ebias = consts.tile([128, 1], F32, tag="ebias")
            nc.vector.memset(ebias, EBIAS)

            # ---- GroupNorm, per-chunk so xn[cc] unblocks as x[cc] lands ----
            for cc in range(CC):
                st = small.tile([128, 2, 6], F32, tag="gn_st", name=f"gn_st{cc}")
                nc.vector.bn_stats(out=st[:, 0, :], in_=x_sb[:, cc, 0:512])
                nc.vector.bn_stats(out=st[:, 1, :], in_=x_sb[:, cc, 512:1024])
                mv = small.tile([128, 2], F32, tag="gn_mv", name=f"gn_mv{cc}")
                nc.vector.bn_aggr(out=mv, in_=st)
                scr = small.tile([128, 1], F32, tag="gn_scr", name=f"gn_scr{cc}")
                nc.vector.tensor_mul(out=scr, in0=mv[:, 0:1], in1=mv[:, 0:1])
                nc.vector.tensor_add(out=mv[:, 1:2], in0=mv[:, 1:2], in1=scr)
                pg = ps_av.tile([8, 2], F32, tag="avB", bufs=1, name=f"gn_pg{cc}")
                nc.tensor.matmul(out=pg, lhsT=ind, rhs=mv, start=True, stop=True)
                # sg cols: [mean_g, ex2->rstd_g, vpe]
                sg = small.tile([8, 4], F32, tag="gn_sg", name=f"gn_sg{cc}")
                nc.vector.tensor_copy(out=sg[:, 0:2], in_=pg)
                nc.vector.scalar_tensor_tensor(out=sg[:, 2:3], in0=sg[:, 0:1], scalar=-1.0, in1=sg[:, 0:1], op0=ALU.mult, op1=ALU.mult)
                nc.vector.scalar_tensor_tensor(out=sg[:, 2:3], in0=sg[:, 1:2], scalar=EPS, in1=sg[:, 2:3], op0=ALU.add, op1=ALU.add)
                nc.scalar.activation(out=sg[:, 3:4], in_=sg[:, 2:3], func=AF.Sqrt, bias=0.0, scale=1.0)
                nc.vector.reciprocal(out=sg[:, 1:2], in_=sg[:, 3:4])
                pbc = ps_av.tile([128, 2], F32, tag="avB", bufs=1, name=f"gn_pbc{cc}")
                nc.tensor.matmul(out=pbc, lhsT=indt, rhs=sg[:, 0:2], start=True, stop=True)
                ab = small.tile([128, 2], F32, tag="gn_ab", name=f"gn_ab{cc}")
                nc.vector.tensor_mul(out=ab[:, 0:1], in0=pbc[:, 1:2], in1=gnw[:, cc:cc + 1])
                nc.vector.scalar_tensor_tensor(out=ab[:, 1:2], in0=pbc[:, 0:1], scalar=-1.0, in1=ab[:, 0:1], op0=ALU.mult, op1=ALU.mult)
                nc.vector.tensor_add(out=ab[:, 1:2], in0=gnb[:, cc:cc + 1], in1=ab[:, 1:2])
                nc.vector.tensor_scalar(out=xn_sb[:, cc, :], in0=x_sb[:, cc, :], scalar1=ab[:, 0:1], scalar2=ab[:, 1:2], op0=ALU.mult, op1=ALU.add)

            # ---- q/k production ----
            def make_qk(oc, pool, tag, bias_on_scalar=False):
                pq = pool.tile([128, HW], F32, tag=tag, bufs=None if tag == "ps_s" else 1, name=f"pq{oc}")
                for cc in range(CC):
                    for q2 in range(QC):
                        nc.tensor.matmul(
                            out=pq[:, q2 * 512:(q2 + 1) * 512],
                            lhsT=qw_sb[:, cc, oc * 128:(oc + 1) * 128],
                            rhs=xn_sb[:, cc, q2 * 512:(q2 + 1) * 512],
                            start=(cc == 0), stop=(cc == CC - 1),
                        )
                if bias_on_scalar:
                    nc.scalar.add(out=qk_sb[:, oc, :], in_=pq[:], add=qkb[:, oc:oc + 1])
                else:
                    nc.vector.tensor_scalar_add(out=qk_sb[:, oc, :], in0=pq[:], scalar1=qkb[:, oc:oc + 1])

            make_qk(0, ps_s, "ps_s", bias_on_scalar=True)
            make_qk(4, ps_s, "ps_s", bias_on_scalar=True)

            # ---- V^T chunks (2 hw-chunks per psum tile), woven into pair 0 ----
            def vt_chunk(i):
                pv = ps_av.tile([128, 2, 512], F32, tag="avB", bufs=1, name=f"pv{i}")
                for h2 in range(2):
                    hwc = 2 * i + h2
                    for cc in range(CC):
                        nc.tensor.matmul(
                            out=pv[:, h2, :],
                            lhsT=xn_sb[:, cc, hwc * 128:(hwc + 1) * 128],
                            rhs=qw_sb[:, cc, 2 * C:3 * C],
                            start=(cc == 0), stop=(cc == CC - 1),
                        )
                for h2 in range(2):
                    hwc = 2 * i + h2
                    nc.vector.tensor_add(
                        out=vt8[:, hwc // 2, hwc % 2, :, 0:HD],
                        in0=pv[:, h2, :].rearrange("p (h d) -> p h d", d=HD),
                        in1=vbb[:].rearrange("p (h d) -> p h d", d=HD),
                    )

            # ---- attention ----
            def normalize_head(hp, a, av_tile):
                avs = small.tile([65, HW], F32, tag="avs", name=f"avs{hp}_{a}")
                nc.vector.tensor_copy(out=avs, in_=av_tile[:, :])
                # denominator row -> [128, 8] (direct SBUF reshape DMA) so the
                # reciprocal runs across all partitions, then DRAM-bounce the
                # bf16 reciprocal out to a [64, 1024] broadcast
                eng = nc.gpsimd if a == 1 else nc.sync
                dt = small.tile([128, 8], F32, tag="dt", name=f"dt{hp}_{a}")
                eng.dma_start(out=dt, in_=avs[64:65, :])
                dtb = small.tile([128, 8], BF16, tag="dtb", name=f"dtb{hp}_{a}")
                with nc.allow_low_precision(reason="bf16 softmax denominator broadcast"):
                    nc.vector.reciprocal(out=dtb, in_=dt)
                rr = drp.tile([HW], BF16, tag="rr", name=f"rr{hp}_{a}")
                eng.dma_start(out=rr, in_=dtb)
                sbc = small.tile([64, HW], BF16, tag="sbc", name=f"sbc{hp}_{a}")
                eng.dma_start(
                    out=sbc,
                    in_=bass.AP(tensor=rr.tensor, offset=rr.offset, ap=[[0, 64]] + list(rr.ap)),
                )
                # partition-shifted DVE write places the odd head at 64:128
                nc.vector.tensor_mul(out=att_t[hp][a * 64:(a + 1) * 64, :], in0=avs[0:64, :], in1=sbc)

            def av_dr(av_tile, head, E_tile, a, kcp, start, stop):
                # fp8 DoubleRow: contracts k-chunks 2*kcp and 2*kcp+1 in one pass
                for q2 in range(QC):
                    w = slice(q2 * 512, (q2 + 1) * 512)
                    nc.tensor.matmul(
                        out=av_tile[:, w],
                        lhsT=vt8[:, kcp, :, head, 0:HD + 1],
                        rhs=E_tile[:, a, 2 * kcp:2 * kcp + 2, w],
                        start=start, stop=stop, perf_mode=DR,
                    )

            E_prev = None
            av0_prev = None

            for hp in range(4):
                E = epool.tile([128, 2, 8, HW], FP8, tag="E", name=f"E{hp}")
                av1 = ps_av.tile([65, HW], F32, tag="avA", bufs=1, name=f"av1_{hp}")
                av0_cur = None
                if hp == 3:
                    av0_cur = ps_av.tile([65, HW], F32, tag="avB", bufs=1, name="av0_3")
                for kc in range(8):
                    if hp == 0 and kc % 2 == 0:
                        vt_chunk(kc // 2)
                    psA = ps_s.tile([128, HW], F32, tag="ps_s", name=f"s{hp}_{kc}_1")
                    psB = ps_s.tile([128, HW], F32, tag="ps_s", name=f"s{hp}_{kc}_0")
                    for q2 in range(QC):
                        w = slice(q2 * 512, (q2 + 1) * 512)
                        nc.tensor.matmul(
                            out=psA[:, w],
                            lhsT=qk_sb[64:128, 4 + hp, kc * 128:(kc + 1) * 128],
                            rhs=qk_sb[64:128, hp, w],
                            start=True, stop=True,
                        )
                        nc.tensor.matmul(
                            out=psB[:, w],
                            lhsT=qk_sb[0:64, 4 + hp, kc * 128:(kc + 1) * 128],
                            rhs=qk_sb[0:64, hp, w],
                            start=True, stop=True,
                        )
                    nc.scalar.activation(out=E[:, 1, kc, :], in_=psA[:], func=AF.Exp, scale=SCALE, bias=ebias[:, 0:1])
                    nc.scalar.activation(out=E[:, 0, kc, :], in_=psB[:], func=AF.Exp, scale=SCALE, bias=ebias[:, 0:1])
                    # head a=1 AV: one DoubleRow pass per completed kc pair
                    if kc % 2 == 1:
                        av_dr(av1, 2 * hp + 1, E, 1, kc // 2, start=(kc == 1), stop=(kc == 7))
                    if hp < 3:
                        # previous pair's a=0 AV spread over kc 1..4
                        if E_prev is not None and 1 <= kc <= 4:
                            av_dr(av0_prev, 2 * (hp - 1), E_prev, 0, kc - 1,
                                  start=(kc == 1), stop=(kc == 4))
                            if kc == 4:
                                normalize_head(hp - 1, 0, av0_prev)
                        if kc == 4:
                            make_qk(hp + 1, ps_av, "avB")
                        if kc == 6:
                            make_qk(4 + hp + 1, ps_av, "avB")
                    else:
                        # last pair: finish pair 2's a=0 early, stream own a=0
                        if kc in (0, 1):
                            for bkcp in (2 * kc, 2 * kc + 1):
                                av_dr(av0_prev, 4, E_prev, 0, bkcp,
                                      start=(bkcp == 0), stop=(bkcp == 3))
                            if kc == 1:
                                normalize_head(2, 0, av0_prev)
                        if kc in (5, 7):
                            av_dr(av0_cur, 6, E, 0, kc // 2, start=(kc == 5), stop=False)
                if hp == 3:
                    for bkcp in (0, 1):
                        av_dr(av0_cur, 6, E, 0, bkcp, start=False, stop=(bkcp == 1))
                normalize_head(hp, 1, av1)
                if hp == 3:
                    normalize_head(3, 0, av0_cur)
                E_prev = E
                if hp < 3:
                    av0_prev = ps_av.tile([65, HW], F32, tag="avB", bufs=1, name=f"av0_{hp}")

            # ---- proj + residual tail ----
            def proj_mm(py, oc, cc):
                for q2 in range(QC):
                    nc.tensor.matmul(
                        out=py[:, q2 * 512:(q2 + 1) * 512],
                        lhsT=pw_sb[:, cc, oc * 128:(oc + 1) * 128],
                        rhs=att_t[cc][:, q2 * 512:(q2 + 1) * 512],
                        start=(cc == 0), stop=(cc == CC - 1),
                    )

            def proj_epilogue(py, oc):
                ot = outp.tile([128, HW], F32, tag="ot", name=f"ot{oc}")
                nc.vector.scalar_tensor_tensor(out=ot, in0=py[:], scalar=pb[:, oc:oc + 1], in1=x_sb[:, oc, :], op0=ALU.add, op1=ALU.add)
                nc.sync.dma_start(out=out_d.ap()[oc * 128:(oc + 1) * 128, :], in_=ot)

            pys = []
            for oc, (pool, tag) in enumerate(
                ((ps_s, "ps_s"), (ps_s, "ps_s"), (ps_av, "avA"), (ps_av, "avB"))
            ):
                py = pool.tile([128, HW], F32, tag=tag, bufs=None if tag == "ps_s" else 1, name=f"py{oc}")
                pys.append(py)
                for cc in range(CC - 1):
                    proj_mm(py, oc, cc)
            for oc in range(4):
                proj_mm(pys[oc], oc, CC - 1)
                proj_epilogue(pys[oc], oc)

    nc.compile()
    return nc


_NC_CACHE = None


def _get_nc():
    global _NC_CACHE
    if _NC_CACHE is None:
        _NC_CACHE = _build()
    return _NC_CACHE


def _prep_in_maps(inputs):
    x = np.asarray(inputs["x"], np.float32)
    gn_w = np.asarray(inputs["gn_w"], np.float32)
    gn_b = np.asarray(inputs["gn_b"], np.float32)
    qkv_w = np.asarray(inputs["qkv_w"], np.float32)
    qkv_b = np.asarray(inputs["qkv_b"], np.float32)
    proj_w = np.asarray(inputs["proj_w"], np.float32)
    proj_b = np.asarray(inputs["proj_b"], np.float32)

    B = x.shape[0]
    xr = x.reshape(B, C, HW).astype(ml_dtypes.bfloat16)
    qwT = np.ascontiguousarray(qkv_w.T).astype(ml_dtypes.bfloat16)
    pwT = np.ascontiguousarray(proj_w.T).astype(ml_dtypes.bfloat16)
    qkb = np.ascontiguousarray(qkv_b[: 2 * C].reshape(8, 128).T)
    vbb = np.ascontiguousarray(np.broadcast_to(qkv_b[2 * C:], (128, C)))
    pb = np.ascontiguousarray(proj_b.reshape(4, 128).T)
    gnw = np.ascontiguousarray(gn_w.reshape(4, 128).T)
    gnb = np.ascontiguousarray(gn_b.reshape(4, 128).T)
    indm = np.zeros((128, 8), np.float32)
    indm[np.arange(128), np.arange(128) // GSIZE] = 1.0 / GSIZE
    ind01 = np.zeros((128, 8), np.float32)
    ind01[np.arange(128), np.arange(128) // GSIZE] = 1.0
    indt = np.ascontiguousarray(ind01.T)
    shared = dict(qw=qwT, pw=pwT, qkb=qkb, vbb=vbb, pb=pb, gnw=gnw, gnb=gnb, ind=indm, indt=indt)
    return [dict(x=np.ascontiguousarray(xr[b]), **shared) for b in range(B)]


def kernel(**inputs):
    nc = _get_nc()
    in_maps = _prep_in_maps(inputs)
    res = run_bass_kernel_spmd(nc, in_maps, core_ids=list(range(8)))
    out = np.stack([r["out"] for r in res.results])
    return out.reshape(8, C, 32, 32).astype(np.float32)


def run_profiled(inputs):
    """kernel() + NTFF profiling; returns (output, exec_time_ns, trace_path)."""
    import types

    import antenv

    if "antenv.axon_hooks" not in sys.modules:
        hooks_mod = types.ModuleType("antenv.axon_hooks")
        _hook = [None]
        hooks_mod.set_axon_ntff_profile_hook = lambda h: _hook.__setitem__(0, h)
        hooks_mod.get_axon_ntff_profile_hook = lambda: _hook[0]
        sys.modules["antenv.axon_hooks"] = hooks_mod
        antenv.axon_hooks = hooks_mod
        from trn_agent_boot.trn_boot import _ntff_profile_via_ctypes

        hooks_mod.set_axon_ntff_profile_hook(_ntff_profile_via_ctypes("/opt/axon/libaxon_pjrt.so"))

    nc = _get_nc()
    in_maps = _prep_in_maps(inputs)
    res = run_bass_kernel_spmd(nc, in_maps, core_ids=list(range(8)), trace=True)
    out = np.stack([r["out"] for r in res.results]).reshape(8, C, 32, 32).astype(np.float32)
    trace = res.instructions_and_trace[1] if res.instructions_and_trace else None
    return out, res.exec_time_ns, trace


# revision 13
# speedup vs baseline: 1.2793x; 1.0097x over previous
"""AttentionBlock Trainium2 kernel: GroupNorm -> QKV -> MHA -> proj -> residual.

Data-parallel over batch B=8 across 8 NeuronCores (one batch image per core).
QKV/proj/score matmuls run in bf16 (fp32 PSUM accumulation); the
attention-value matmuls run in fp8 e4m3 DoubleRow (2 k-chunks per pass):
E is stored as exp(s*scale)/16 in e4m3 (the /16 keeps e4m3 in range; it
rides both the numerator and the ones-column denominator so it cancels).

Per-core layouts (C=512 channels, HW=1024 tokens, 8 heads, hd=64):
  x            [C, HW] bf16 (stats + residual tolerate bf16)
  xn           [C, HW] bf16, per-128-channel-chunk GroupNorm
  q, k         qk_sb[128, 8 oc, 1024] bf16; pair hp: oc=hp (q), 4+hp (k),
               head a at partitions a*64:(a+1)*64
  vT (fp8)     vt8[128, kcp 4, ko 2, head 8, 68] (65 used: 64 + ones col)
  scoresT      [k, q] PSUM [128, 1024] per (head, kc)
  E            [128, 2 head, 8 kc, 1024] fp8 per pair
  att          [C, HW] bf16; out [C, HW] fp32 = proj(att) + pb + x

PSUM (8 banks): ps_s x2 (scores / lead-in qk / proj py0-1) = 4 banks;
avA x1 (in-pair AV accumulator / py2) + avB x1 (rotating: GN stats, V^T,
mid-pair qk, deferred a=0 AV, py3) = 4 banks.

Softmax denominators: AV psum row 64 -> SBUF; reciprocal_approx_fast on the
row; DRAM bounce broadcast to [64, 1024]; multiply (partition-shifted write
for the odd head).
"""

import sys

if "/opt/trn_rl_repo" not in sys.path:
    sys.path.insert(0, "/opt/trn_rl_repo")

import numpy as np
import ml_dtypes

import concourse.bass as bass
import concourse.tile as tile
from concourse import mybir, bacc
from concourse.bass_utils import run_bass_kernel_spmd

AF = mybir.ActivationFunctionType
ALU = mybir.AluOpType
F32 = mybir.dt.float32
BF16 = mybir.dt.bfloat16
FP8 = mybir.dt.float8e4
DR = mybir.MatmulPerfMode.DoubleRow

C = 512
HW = 1024
NHEADS = 8
HD = 64
HDP = 68  # padded head stride so the DoubleRow ko-step (8*HDP) is 16-aligned
NGROUPS = 32
GSIZE = 16
EPS = 1e-5
SCALE = HD ** -0.5
EBIAS = -4.0 * float(np.log(2.0))  # exp output scaled by 1/16 for e4m3 range
CC = 4
OCQK = 8
QC = 2


def _build():
    nc = bacc.Bacc("TRN2", target_bir_lowering=False, debug=False, num_devices=8)

    x_d = nc.dram_tensor("x", [C, HW], BF16, kind="ExternalInput")
    qw_d = nc.dram_tensor("qw", [C, 3 * C], BF16, kind="ExternalInput")
    pw_d = nc.dram_tensor("pw", [C, C], BF16, kind="ExternalInput")
    qkb_d = nc.dram_tensor("qkb", [128, 8], F32, kind="ExternalInput")
    vbb_d = nc.dram_tensor("vbb", [128, C], F32, kind="ExternalInput")
    pb_d = nc.dram_tensor("pb", [128, 4], F32, kind="ExternalInput")
    gnw_d = nc.dram_tensor("gnw", [128, 4], F32, kind="ExternalInput")
    gnb_d = nc.dram_tensor("gnb", [128, 4], F32, kind="ExternalInput")
    ind_d = nc.dram_tensor("ind", [128, 8], F32, kind="ExternalInput")
    indt_d = nc.dram_tensor("indt", [8, 128], F32, kind="ExternalInput")
    out_d = nc.dram_tensor("out", [C, HW], F32, kind="ExternalOutput")

    with tile.TileContext(nc) as tc:
        with (
            tc.tile_pool(name="consts", bufs=1) as consts,
            tc.tile_pool(name="epool", bufs=2) as epool,
            tc.tile_pool(name="small", bufs=4) as small,
            tc.tile_pool(name="outp", bufs=3) as outp,
            tc.tile_pool(name="drp", bufs=4, space="DRAM") as drp,
            tc.tile_pool(name="ps_s", bufs=2, space="PSUM") as ps_s,
            tc.tile_pool(name="ps_av", bufs=1, space="PSUM") as ps_av,
        ):
            # ---- input DMAs: x chunks first (GN gates on them), then tables ----
            x_sb = consts.tile([128, CC, HW], BF16, tag="x")
            x_r = x_d.ap().rearrange("(cc p) hw -> p cc hw", p=128)
            for cc in range(CC):
                nc.sync.dma_start(out=x_sb[:, cc, :], in_=x_r[:, cc, :])
            ind = consts.tile([128, 8], F32, tag="ind")
            nc.gpsimd.dma_start(out=ind, in_=ind_d.ap())
            indt = consts.tile([8, 128], F32, tag="indt")
            nc.gpsimd.dma_start(out=indt, in_=indt_d.ap())
            gnw = consts.tile([128, 4], F32, tag="gnw")
            nc.gpsimd.dma_start(out=gnw, in_=gnw_d.ap())
            gnb = consts.tile([128, 4], F32, tag="gnb")
            nc.gpsimd.dma_start(out=gnb, in_=gnb_d.ap())
            qkb = consts.tile([128, 8], F32, tag="qkb")
            nc.gpsimd.dma_start(out=qkb, in_=qkb_d.ap())
            vbb = consts.tile([128, C], F32, tag="vbb")
            nc.gpsimd.dma_start(out=vbb, in_=vbb_d.ap())
            pb = consts.tile([128, 4], F32, tag="pb")
            nc.gpsimd.dma_start(out=pb, in_=pb_d.ap())
            qw_sb = consts.tile([128, CC, 3 * C], BF16, tag="qw")
            qw_r = qw_d.ap().rearrange("(cc p) o -> p cc o", p=128)
            nc.sync.dma_start(out=qw_sb[:, :, 0:2 * C], in_=qw_r[:, :, 0:2 * C])
            nc.sync.dma_start(out=qw_sb[:, :, 2 * C:3 * C], in_=qw_r[:, :, 2 * C:3 * C])
            pw_sb = consts.tile([128, CC, C], BF16, tag="pw")
            nc.sync.dma_start(out=pw_sb, in_=pw_d.ap().rearrange("(cc p) o -> p cc o", p=128))

            xn_sb = consts.tile([128, CC, HW], BF16, tag="xn")
            qk_sb = consts.tile([128, OCQK, HW], BF16, tag="qk")
            vt8 = consts.tile([128, 4, 2, NHEADS, HDP], FP8, tag="vt")
            att_t = [consts.tile([128, HW], BF16, tag=f"att{i}", name=f"att{i}") for i in range(CC)]

            # ones column of vT (softmax denominator trick)
            nc.vector.memset(vt8[:, :, :, :, HD:HD + 1], 1.0)
            # exp bias constant (E scaled by 1/16 for e4m3 range)
            ebias = consts.tile([128, 1], F32, tag="ebias")
            nc.vector.memset(ebias, EBIAS)

            # ---- GroupNorm, per-chunk so xn[cc] unblocks as x[cc] lands ----
            for cc in range(CC):
                st = small.tile([128, 2, 6], F32, tag="gn_st", name=f"gn_st{cc}")
                nc.vector.bn_stats(out=st[:, 0, :], in_=x_sb[:, cc, 0:512])
                nc.vector.bn_stats(out=st[:, 1, :], in_=x_sb[:, cc, 512:1024])
                mv = small.tile([128, 2], F32, tag="gn_mv", name=f"gn_mv{cc}")
                nc.vector.bn_aggr(out=mv, in_=st)
                scr = small.tile([128, 1], F32, tag="gn_scr", name=f"gn_scr{cc}")
                nc.vector.tensor_mul(out=scr, in0=mv[:, 0:1], in1=mv[:, 0:1])
                nc.vector.tensor_add(out=mv[:, 1:2], in0=mv[:, 1:2], in1=scr)
                pg = ps_s.tile([8, 2], F32, tag="ps_s", name=f"gn_pg{cc}")
                nc.tensor.matmul(out=pg, lhsT=ind, rhs=mv, start=True, stop=True)
                # sg cols: [mean_g, ex2->rstd_g, vpe]
                sg = small.tile([8, 4], F32, tag="gn_sg", name=f"gn_sg{cc}")
                nc.vector.tensor_copy(out=sg[:, 0:2], in_=pg)
                nc.vector.scalar_tensor_tensor(out=sg[:, 2:3], in0=sg[:, 0:1], scalar=-1.0, in1=sg[:, 0:1], op0=ALU.mult, op1=ALU.mult)
                nc.vector.scalar_tensor_tensor(out=sg[:, 2:3], in0=sg[:, 1:2], scalar=EPS, in1=sg[:, 2:3], op0=ALU.add, op1=ALU.add)
                nc.scalar.activation(out=sg[:, 3:4], in_=sg[:, 2:3], func=AF.Sqrt, bias=0.0, scale=1.0)
                nc.vector.reciprocal(out=sg[:, 1:2], in_=sg[:, 3:4])
                pbc = ps_s.tile([128, 2], F32, tag="ps_s", name=f"gn_pbc{cc}")
                nc.tensor.matmul(out=pbc, lhsT=indt, rhs=sg[:, 0:2], start=True, stop=True)
                ab = small.tile([128, 2], F32, tag="gn_ab", name=f"gn_ab{cc}")
                nc.vector.tensor_mul(out=ab[:, 0:1], in0=pbc[:, 1:2], in1=gnw[:, cc:cc + 1])
                nc.vector.scalar_tensor_tensor(out=ab[:, 1:2], in0=pbc[:, 0:1], scalar=-1.0, in1=ab[:, 0:1], op0=ALU.mult, op1=ALU.mult)
                nc.vector.tensor_add(out=ab[:, 1:2], in0=gnb[:, cc:cc + 1], in1=ab[:, 1:2])
                nc.vector.tensor_scalar(out=xn_sb[:, cc, :], in0=x_sb[:, cc, :], scalar1=ab[:, 0:1], scalar2=ab[:, 1:2], op0=ALU.mult, op1=ALU.add)

            # ---- q/k production ----
            def make_qk(oc, pool, tag, bias_on_scalar=False):
                pq = pool.tile([128, HW], F32, tag=tag, bufs=None if tag == "ps_s" else 1, name=f"pq{oc}")
                for cc in range(CC):
                    for q2 in range(QC):
                        nc.tensor.matmul(
                            out=pq[:, q2 * 512:(q2 + 1) * 512],
                            lhsT=qw_sb[:, cc, oc * 128:(oc + 1) * 128],
                            rhs=xn_sb[:, cc, q2 * 512:(q2 + 1) * 512],
                            start=(cc == 0), stop=(cc == CC - 1),
                        )
                if bias_on_scalar:
                    nc.scalar.add(out=qk_sb[:, oc, :], in_=pq[:], add=qkb[:, oc:oc + 1])
                else:
                    nc.vector.tensor_scalar_add(out=qk_sb[:, oc, :], in0=pq[:], scalar1=qkb[:, oc:oc + 1])

            make_qk(0, ps_s, "ps_s", bias_on_scalar=True)
            make_qk(4, ps_s, "ps_s", bias_on_scalar=True)

            # ---- V^T chunks (2 hw-chunks per psum tile), woven into pair 0 ----
            def vt_chunk(i):
                pv = ps_av.tile([128, 2, 512], F32, tag="avB", bufs=1, name=f"pv{i}")
                for h2 in range(2):
                    hwc = 2 * i + h2
                    for cc in range(CC):
                        nc.tensor.matmul(
                            out=pv[:, h2, :],
                            lhsT=xn_sb[:, cc, hwc * 128:(hwc + 1) * 128],
                            rhs=qw_sb[:, cc, 2 * C:3 * C],
                            start=(cc == 0), stop=(cc == CC - 1),
                        )
                for h2 in range(2):
                    hwc = 2 * i + h2
                    nc.vector.tensor_add(
                        out=vt8[:, hwc // 2, hwc % 2, :, 0:HD],
                        in0=pv[:, h2, :].rearrange("p (h d) -> p h d", d=HD),
                        in1=vbb[:].rearrange("p (h d) -> p h d", d=HD),
                    )

            # ---- attention ----
            def normalize_head(hp, a, av_tile):
                avs = small.tile([65, HW], F32, tag="avs", name=f"avs{hp}_{a}")
                nc.vector.tensor_copy(out=avs, in_=av_tile[:, :])
                # denominator row -> [128, 8] (direct SBUF reshape DMA) so the
                # reciprocal runs across all partitions, then DRAM-bounce the
                # bf16 reciprocal out to a [64, 1024] broadcast
                eng = nc.gpsimd if a == 1 else nc.sync
                dt = small.tile([128, 8], F32, tag="dt", name=f"dt{hp}_{a}")
                eng.dma_start(out=dt, in_=avs[64:65, :])
                dtb = small.tile([128, 8], BF16, tag="dtb", name=f"dtb{hp}_{a}")
                with nc.allow_low_precision(reason="bf16 softmax denominator broadcast"):
                    nc.vector.reciprocal(out=dtb, in_=dt)
                rr = drp.tile([HW], BF16, tag="rr", name=f"rr{hp}_{a}")
                eng.dma_start(out=rr, in_=dtb)
                sbc = small.tile([64, HW], BF16, tag="sbc", name=f"sbc{hp}_{a}")
                eng.dma_start(
                    out=sbc,
                    in_=bass.AP(tensor=rr.tensor, offset=rr.offset, ap=[[0, 64]] + list(rr.ap)),
                )
                # partition-shifted DVE write places the odd head at 64:128
                nc.vector.tensor_mul(out=att_t[hp][a * 64:(a + 1) * 64, :], in0=avs[0:64, :], in1=sbc)

            def av_dr(av_tile, head, E_tile, a, kcp, start, stop):
                # fp8 DoubleRow: contracts k-chunks 2*kcp and 2*kcp+1 in one pass
                for q2 in range(QC):
                    w = slice(q2 * 512, (q2 + 1) * 512)
                    nc.tensor.matmul(
                        out=av_tile[:, w],
                        lhsT=vt8[:, kcp, :, head, 0:HD + 1],
                        rhs=E_tile[:, a, 2 * kcp:2 * kcp + 2, w],
                        start=start, stop=stop, perf_mode=DR,
                    )

            E_prev = None
            av0_prev = None

            for hp in range(4):
                E = epool.tile([128, 2, 8, HW], FP8, tag="E", name=f"E{hp}")
                av1 = ps_av.tile([65, HW], F32, tag="avA", bufs=1, name=f"av1_{hp}")
                av0_cur = None
                if hp == 3:
                    av0_cur = ps_av.tile([65, HW], F32, tag="avB", bufs=1, name="av0_3")
                for kc in range(8):
                    if hp == 0 and kc % 2 == 0:
                        vt_chunk(kc // 2)
                    psA = ps_s.tile([128, HW], F32, tag="ps_s", name=f"s{hp}_{kc}_1")
                    psB = ps_s.tile([128, HW], F32, tag="ps_s", name=f"s{hp}_{kc}_0")
                    for q2 in range(QC):
                        w = slice(q2 * 512, (q2 + 1) * 512)
                        nc.tensor.matmul(
                            out=psA[:, w],
                            lhsT=qk_sb[64:128, 4 + hp, kc * 128:(kc + 1) * 128],
                            rhs=qk_sb[64:128, hp, w],
                            start=True, stop=True,
                        )
                        nc.tensor.matmul(
                            out=psB[:, w],
                            lhsT=qk_sb[0:64, 4 + hp, kc * 128:(kc + 1) * 128],
                            rhs=qk_sb[0:64, hp, w],
                            start=True, stop=True,
                        )
                    nc.scalar.activation(out=E[:, 1, kc, :], in_=psA[:], func=AF.Exp, scale=SCALE, bias=ebias[:, 0:1])
                    nc.scalar.activation(out=E[:, 0, kc, :], in_=psB[:], func=AF.Exp, scale=SCALE, bias=ebias[:, 0:1])
                    # head a=1 AV: one DoubleRow pass per completed kc pair
                    if kc % 2 == 1:
                        av_dr(av1, 2 * hp + 1, E, 1, kc // 2, start=(kc == 1), stop=(kc == 7))
                    if hp < 3:
                        # previous pair's a=0 AV spread over kc 1..4
                        if E_prev is not None and 1 <= kc <= 4:
                            av_dr(av0_prev, 2 * (hp - 1), E_prev, 0, kc - 1,
                                  start=(kc == 1), stop=(kc == 4))
                            if kc == 4:
                                normalize_head(hp - 1, 0, av0_prev)
                        if kc == 4:
                            make_qk(hp + 1, ps_av, "avB")
                        if kc == 6:
                            make_qk(4 + hp + 1, ps_av, "avB")
                    else:
                        # last pair: finish pair 2's a=0 early, stream own a=0
                        if kc in (0, 1):
                            for bkcp in (2 * kc, 2 * kc + 1):
                                av_dr(av0_prev, 4, E_prev, 0, bkcp,
                                      start=(bkcp == 0), stop=(bkcp == 3))
                            if kc == 1:
                                normalize_head(2, 0, av0_prev)
                        if kc in (5, 7):
                            av_dr(av0_cur, 6, E, 0, kc // 2, start=(kc == 5), stop=False)
                if hp == 3:
                    for bkcp in (0, 1):
                        av_dr(av0_cur, 6, E, 0, bkcp, start=False, stop=(bkcp == 1))
                normalize_head(hp, 1, av1)
                if hp == 3:
                    normalize_head(3, 0, av0_cur)
                E_prev = E
                if hp < 3:
                    av0_prev = ps_av.tile([65, HW], F32, tag="avB", bufs=1, name=f"av0_{hp}")

            # ---- proj + residual tail ----
            def proj_mm(py, oc, cc):
                for q2 in range(QC):
                    nc.tensor.matmul(
                        out=py[:, q2 * 512:(q2 + 1) * 512],
                        lhsT=pw_sb[:, cc, oc * 128:(oc + 1) * 128],
                        rhs=att_t[cc][:, q2 * 512:(q2 + 1) * 512],
                        start=(cc == 0), stop=(cc == CC - 1),
                    )

            def proj_epilogue(py, oc):
                ot = outp.tile([128, HW], F32, tag="ot", name=f"ot{oc}")
                nc.vector.scalar_tensor_tensor(out=ot, in0=py[:], scalar=pb[:, oc:oc + 1], in1=x_sb[:, oc, :], op0=ALU.add, op1=ALU.add)
                nc.sync.dma_start(out=out_d.ap()[oc * 128:(oc + 1) * 128, :], in_=ot)

            pys = []
            for oc, (pool, tag) in enumerate(
                ((ps_s, "ps_s"), (ps_s, "ps_s"), (ps_av, "avA"), (ps_av, "avB"))
            ):
                py = pool.tile([128, HW], F32, tag=tag, bufs=None if tag == "ps_s" else 1, name=f"py{oc}")
                pys.append(py)
                for cc in range(CC - 1):
                    proj_mm(py, oc, cc)
            for oc in range(4):
                proj_mm(pys[oc], oc, CC - 1)
                proj_epilogue(pys[oc], oc)

    nc.compile()
    return nc


_NC_CACHE = None


def _get_nc():
    global _NC_CACHE
    if _NC_CACHE is None:
        _NC_CACHE = _build()
    return _NC_CACHE


def _prep_in_maps(inputs):
    x = np.asarray(inputs["x"], np.float32)
    gn_w = np.asarray(inputs["gn_w"], np.float32)
    gn_b = np.asarray(inputs["gn_b"], np.float32)
    qkv_w = np.asarray(inputs["qkv_w"], np.float32)
    qkv_b = np.asarray(inputs["qkv_b"], np.float32)
    proj_w = np.asarray(inputs["proj_w"], np.float32)
    proj_b = np.asarray(inputs["proj_b"], np.float32)

    B = x.shape[0]
    xr = x.reshape(B, C, HW).astype(ml_dtypes.bfloat16)
    qwT = np.ascontiguousarray(qkv_w.T).astype(ml_dtypes.bfloat16)
    pwT = np.ascontiguousarray(proj_w.T).astype(ml_dtypes.bfloat16)
    qkb = np.ascontiguousarray(qkv_b[: 2 * C].reshape(8, 128).T)
    vbb = np.ascontiguousarray(np.broadcast_to(qkv_b[2 * C:], (128, C)))
    pb = np.ascontiguousarray(proj_b.reshape(4, 128).T)
    gnw = np.ascontiguousarray(gn_w.reshape(4, 128).T)
    gnb = np.ascontiguousarray(gn_b.reshape(4, 128).T)
    indm = np.zeros((128, 8), np.float32)
    indm[np.arange(128), np.arange(128) // GSIZE] = 1.0 / GSIZE
    ind01 = np.zeros((128, 8), np.float32)
    ind01[np.arange(128), np.arange(128) // GSIZE] = 1.0
    indt = np.ascontiguousarray(ind01.T)
    shared = dict(qw=qwT, pw=pwT, qkb=qkb, vbb=vbb, pb=pb, gnw=gnw, gnb=gnb, ind=indm, indt=indt)
    return [dict(x=np.ascontiguousarray(xr[b]), **shared) for b in range(B)]


def kernel(**inputs):
    nc = _get_nc()
    in_maps = _prep_in_maps(inputs)
    res = run_bass_kernel_spmd(nc, in_maps, core_ids=list(range(8)))
    out = np.stack([r["out"] for r in res.results])
    return out.reshape(8, C, 32, 32).astype(np.float32)


def run_profiled(inputs):
    """kernel() + NTFF profiling; returns (output, exec_time_ns, trace_path)."""
    import types

    import antenv

    if "antenv.axon_hooks" not in sys.modules:
        hooks_mod = types.ModuleType("antenv.axon_hooks")
        _hook = [None]
        hooks_mod.set_axon_ntff_profile_hook = lambda h: _hook.__setitem__(0, h)
        hooks_mod.get_axon_ntff_profile_hook = lambda: _hook[0]
        sys.modules["antenv.axon_hooks"] = hooks_mod
        antenv.axon_hooks = hooks_mod
        from trn_agent_boot.trn_boot import _ntff_profile_via_ctypes

        hooks_mod.set_axon_ntff_profile_hook(_ntff_profile_via_ctypes("/opt/axon/libaxon_pjrt.so"))

    nc = _get_nc()
    in_maps = _prep_in_maps(inputs)
    res = run_bass_kernel_spmd(nc, in_maps, core_ids=list(range(8)), trace=True)
    out = np.stack([r["out"] for r in res.results]).reshape(8, C, 32, 32).astype(np.float32)
    trace = res.instructions_and_trace[1] if res.instructions_and_trace else None
    return out, res.exec_time_ns, trace


# revision 15
# speedup vs baseline: 1.3121x; 1.0257x over previous
"""AttentionBlock Trainium2 kernel: GroupNorm -> QKV -> MHA -> proj -> residual.

Data-parallel over batch B=8 across 8 NeuronCores (one batch image per core).
QKV/proj/score matmuls run in bf16 (fp32 PSUM accumulation); the
attention-value matmuls run in fp8 e4m3 DoubleRow (2 k-chunks per pass):
E is stored as exp(s*scale)/16 in e4m3 (the /16 keeps e4m3 in range; it
rides both the numerator and the ones-column denominator so it cancels).

Per-core layouts (C=512 channels, HW=1024 tokens, 8 heads, hd=64):
  x            [C, HW] bf16 (stats + residual tolerate bf16)
  xn           [C, HW] bf16, per-128-channel-chunk GroupNorm
  q, k         qk_sb[128, 8 oc, 1024] bf16; pair hp: oc=hp (q), 4+hp (k),
               head a at partitions a*64:(a+1)*64
  vT (fp8)     vt8[128, kcp 4, ko 2, head 8, 68] (65 used: 64 + ones col)
  scoresT      [k, q] PSUM [128, 1024] per (head, kc)
  E            [128, 2 head, 8 kc, 1024] fp8 per pair
  att          [C, HW] bf16; out [C, HW] fp32 = proj(att) + pb + x

PSUM (8 banks): ps_s x2 (scores / lead-in qk / proj py0-1) = 4 banks;
avA x1 (in-pair AV accumulator / py2) + avB x1 (rotating: GN stats, V^T,
mid-pair qk, deferred a=0 AV, py3) = 4 banks.

Softmax denominators: AV psum row 64 -> SBUF; reciprocal_approx_fast on the
row; DRAM bounce broadcast to [64, 1024]; multiply (partition-shifted write
for the odd head).
"""

import sys

if "/opt/trn_rl_repo" not in sys.path:
    sys.path.insert(0, "/opt/trn_rl_repo")

import numpy as np
import ml_dtypes

import concourse.bass as bass
import concourse.tile as tile
from concourse import mybir, bacc
from concourse.bass_utils import run_bass_kernel_spmd

AF = mybir.ActivationFunctionType
ALU = mybir.AluOpType
F32 = mybir.dt.float32
BF16 = mybir.dt.bfloat16
FP8 = mybir.dt.float8e4
DR = mybir.MatmulPerfMode.DoubleRow

C = 512
HW = 1024
NHEADS = 8
HD = 64
HDP = 68  # padded head stride so the DoubleRow ko-step (8*HDP) is 16-aligned
NGROUPS = 32
GSIZE = 16
EPS = 1e-5
SCALE = HD ** -0.5
EBIAS = -4.0 * float(np.log(2.0))  # exp output scaled by 1/16 for e4m3 range
CC = 4
OCQK = 8
QC = 2


def _build():
    nc = bacc.Bacc("TRN2", target_bir_lowering=False, debug=False, num_devices=8)

    x_d = nc.dram_tensor("x", [C, HW], BF16, kind="ExternalInput")
    qw_d = nc.dram_tensor("qw", [C, 3 * C], BF16, kind="ExternalInput")
    pw_d = nc.dram_tensor("pw", [C, C], BF16, kind="ExternalInput")
    qkb_d = nc.dram_tensor("qkb", [128, 8], F32, kind="ExternalInput")
    vbb_d = nc.dram_tensor("vbb", [128, C], F32, kind="ExternalInput")
    pb_d = nc.dram_tensor("pb", [128, 4], F32, kind="ExternalInput")
    gnw_d = nc.dram_tensor("gnw", [128, 4], F32, kind="ExternalInput")
    gnb_d = nc.dram_tensor("gnb", [128, 4], F32, kind="ExternalInput")
    ind_d = nc.dram_tensor("ind", [128, 8], F32, kind="ExternalInput")
    indt_d = nc.dram_tensor("indt", [8, 128], F32, kind="ExternalInput")
    out_d = nc.dram_tensor("out", [C, HW], F32, kind="ExternalOutput")

    with tile.TileContext(nc) as tc:
        with (
            tc.tile_pool(name="consts", bufs=1) as consts,
            tc.tile_pool(name="epool", bufs=2) as epool,
            tc.tile_pool(name="small", bufs=4) as small,
            tc.tile_pool(name="outp", bufs=3) as outp,
            tc.tile_pool(name="drp", bufs=4, space="DRAM") as drp,
            tc.tile_pool(name="ps_s", bufs=2, space="PSUM") as ps_s,
            tc.tile_pool(name="ps_av", bufs=1, space="PSUM") as ps_av,
        ):
            # ---- input DMAs: x chunks first (GN gates on them), then tables ----
            x_sb = consts.tile([128, CC, HW], BF16, tag="x")
            x_r = x_d.ap().rearrange("(cc p) hw -> p cc hw", p=128)
            for cc in range(CC):
                nc.sync.dma_start(out=x_sb[:, cc, :], in_=x_r[:, cc, :])
            ind = consts.tile([128, 8], F32, tag="ind")
            nc.gpsimd.dma_start(out=ind, in_=ind_d.ap())
            indt = consts.tile([8, 128], F32, tag="indt")
            nc.gpsimd.dma_start(out=indt, in_=indt_d.ap())
            gnw = consts.tile([128, 4], F32, tag="gnw")
            nc.gpsimd.dma_start(out=gnw, in_=gnw_d.ap())
            gnb = consts.tile([128, 4], F32, tag="gnb")
            nc.gpsimd.dma_start(out=gnb, in_=gnb_d.ap())
            qkb = consts.tile([128, 8], F32, tag="qkb")
            nc.gpsimd.dma_start(out=qkb, in_=qkb_d.ap())
            vbb = consts.tile([128, C], F32, tag="vbb")
            nc.gpsimd.dma_start(out=vbb, in_=vbb_d.ap())
            pb = consts.tile([128, 4], F32, tag="pb")
            nc.gpsimd.dma_start(out=pb, in_=pb_d.ap())
            qw_sb = consts.tile([128, CC, 3 * C], BF16, tag="qw")
            qw_r = qw_d.ap().rearrange("(cc p) o -> p cc o", p=128)
            nc.sync.dma_start(out=qw_sb[:, :, 0:2 * C], in_=qw_r[:, :, 0:2 * C])
            nc.sync.dma_start(out=qw_sb[:, :, 2 * C:3 * C], in_=qw_r[:, :, 2 * C:3 * C])
            pw_sb = consts.tile([128, CC, C], BF16, tag="pw")
            nc.sync.dma_start(out=pw_sb, in_=pw_d.ap().rearrange("(cc p) o -> p cc o", p=128))

            xn_sb = consts.tile([128, CC, HW], BF16, tag="xn")
            qk_sb = consts.tile([128, OCQK, HW], BF16, tag="qk")
            vt8 = consts.tile([128, 4, 2, NHEADS, HDP], FP8, tag="vt")
            att_t = [consts.tile([128, HW], BF16, tag=f"att{i}", name=f"att{i}") for i in range(CC)]

            # ones column of vT (softmax denominator trick)
            nc.vector.memset(vt8[:, :, :, :, HD:HD + 1], 1.0)
            # exp bias constant (E scaled by 1/16 for e4m3 range)
            ebias = consts.tile([128, 1], F32, tag="ebias")
            nc.vector.memset(ebias, EBIAS)

            # ---- GroupNorm: stats stream per chunk, then one batched chain ----
            st = small.tile([128, CC, 2, 6], F32, tag="gn_st")
            for cc in range(CC):
                nc.vector.bn_stats(out=st[:, cc, 0, :], in_=x_sb[:, cc, 0:512])
                nc.vector.bn_stats(out=st[:, cc, 1, :], in_=x_sb[:, cc, 512:1024])
            mv = small.tile([128, CC, 2], F32, tag="gn_mv")
            for cc in range(CC):
                nc.vector.bn_aggr(out=mv[:, cc, :], in_=st[:, cc, :, :])
            scr = small.tile([128, CC, 1], F32, tag="gn_scr")
            nc.vector.tensor_mul(out=scr, in0=mv[:, :, 0:1], in1=mv[:, :, 0:1])
            nc.vector.tensor_add(out=mv[:, :, 1:2], in0=mv[:, :, 1:2], in1=scr)
            pg = ps_s.tile([8, CC, 2], F32, tag="ps_s", name="gn_pg")
            nc.tensor.matmul(out=pg, lhsT=ind, rhs=mv.rearrange("p cc s -> p (cc s)"), start=True, stop=True)
            # sg: [mean, ex2->rstd] contiguous for the broadcast matmul; tt scratch
            sg = small.tile([8, CC, 2], F32, tag="gn_sg")
            tt = small.tile([8, CC, 2], F32, tag="gn_tt")
            nc.vector.tensor_copy(out=sg, in_=pg)
            nc.vector.scalar_tensor_tensor(out=tt[:, :, 0:1], in0=sg[:, :, 0:1], scalar=-1.0, in1=sg[:, :, 0:1], op0=ALU.mult, op1=ALU.mult)
            nc.vector.scalar_tensor_tensor(out=tt[:, :, 0:1], in0=sg[:, :, 1:2], scalar=EPS, in1=tt[:, :, 0:1], op0=ALU.add, op1=ALU.add)
            nc.scalar.activation(out=tt[:, :, 1:2], in_=tt[:, :, 0:1], func=AF.Sqrt, bias=0.0, scale=1.0)
            nc.vector.reciprocal(out=sg[:, :, 1:2], in_=tt[:, :, 1:2])
            pbc = ps_s.tile([128, CC, 2], F32, tag="ps_s", name="gn_pbc")
            nc.tensor.matmul(out=pbc, lhsT=indt, rhs=sg.rearrange("g cc s -> g (cc s)"), start=True, stop=True)
            ab = small.tile([128, CC, 2], F32, tag="gn_ab")
            nc.vector.tensor_mul(out=ab[:, :, 0:1], in0=pbc[:, :, 1:2], in1=gnw.rearrange("p (cc one) -> p cc one", one=1))
            nc.vector.scalar_tensor_tensor(out=ab[:, :, 1:2], in0=pbc[:, :, 0:1], scalar=-1.0, in1=ab[:, :, 0:1], op0=ALU.mult, op1=ALU.mult)
            nc.vector.tensor_add(out=ab[:, :, 1:2], in0=gnb.rearrange("p (cc one) -> p cc one", one=1), in1=ab[:, :, 1:2])
            for cc in range(CC):
                nc.vector.tensor_scalar(out=xn_sb[:, cc, :], in0=x_sb[:, cc, :], scalar1=ab[:, cc, 0:1], scalar2=ab[:, cc, 1:2], op0=ALU.mult, op1=ALU.add)

            # ---- q/k production ----
            def make_qk(oc, pool, tag, bias_on_scalar=False):
                pq = pool.tile([128, HW], F32, tag=tag, bufs=None if tag == "ps_s" else 1, name=f"pq{oc}")
                for cc in range(CC):
                    for q2 in range(QC):
                        nc.tensor.matmul(
                            out=pq[:, q2 * 512:(q2 + 1) * 512],
                            lhsT=qw_sb[:, cc, oc * 128:(oc + 1) * 128],
                            rhs=xn_sb[:, cc, q2 * 512:(q2 + 1) * 512],
                            start=(cc == 0), stop=(cc == CC - 1),
                        )
                if bias_on_scalar:
                    nc.scalar.add(out=qk_sb[:, oc, :], in_=pq[:], add=qkb[:, oc:oc + 1])
                else:
                    nc.vector.tensor_scalar_add(out=qk_sb[:, oc, :], in0=pq[:], scalar1=qkb[:, oc:oc + 1])

            make_qk(0, ps_s, "ps_s", bias_on_scalar=True)
            make_qk(4, ps_s, "ps_s", bias_on_scalar=True)

            # ---- V^T chunks (2 hw-chunks per psum tile), woven into pair 0 ----
            def vt_chunk(i):
                pv = ps_av.tile([128, 2, 512], F32, tag="avB", bufs=1, name=f"pv{i}")
                for h2 in range(2):
                    hwc = 2 * i + h2
                    for cc in range(CC):
                        nc.tensor.matmul(
                            out=pv[:, h2, :],
                            lhsT=xn_sb[:, cc, hwc * 128:(hwc + 1) * 128],
                            rhs=qw_sb[:, cc, 2 * C:3 * C],
                            start=(cc == 0), stop=(cc == CC - 1),
                        )
                for h2 in range(2):
                    hwc = 2 * i + h2
                    nc.vector.tensor_add(
                        out=vt8[:, hwc // 2, hwc % 2, :, 0:HD],
                        in0=pv[:, h2, :].rearrange("p (h d) -> p h d", d=HD),
                        in1=vbb[:].rearrange("p (h d) -> p h d", d=HD),
                    )

            # ---- attention ----
            def normalize_head(hp, a, av_tile):
                avs = small.tile([65, HW], F32, tag="avs", name=f"avs{hp}_{a}")
                nc.vector.tensor_copy(out=avs, in_=av_tile[:, :])
                # denominator row -> [128, 8] (direct SBUF reshape DMA) so the
                # reciprocal runs across all partitions, then DRAM-bounce the
                # bf16 reciprocal out to a [64, 1024] broadcast
                eng = nc.gpsimd if a == 1 else nc.sync
                dt = small.tile([128, 8], F32, tag="dt", name=f"dt{hp}_{a}")
                eng.dma_start(out=dt, in_=avs[64:65, :])
                dtb = small.tile([128, 8], BF16, tag="dtb", name=f"dtb{hp}_{a}")
                with nc.allow_low_precision(reason="bf16 softmax denominator broadcast"):
                    nc.vector.reciprocal(out=dtb, in_=dt)
                rr = drp.tile([HW], BF16, tag="rr", name=f"rr{hp}_{a}")
                eng.dma_start(out=rr, in_=dtb)
                sbc = small.tile([64, HW], BF16, tag="sbc", name=f"sbc{hp}_{a}")
                eng.dma_start(
                    out=sbc,
                    in_=bass.AP(tensor=rr.tensor, offset=rr.offset, ap=[[0, 64]] + list(rr.ap)),
                )
                # partition-shifted DVE write places the odd head at 64:128
                nc.vector.tensor_mul(out=att_t[hp][a * 64:(a + 1) * 64, :], in0=avs[0:64, :], in1=sbc)

            def av_dr(av_tile, head, E_tile, a, kcp, start, stop):
                # fp8 DoubleRow: contracts k-chunks 2*kcp and 2*kcp+1 in one pass
                for q2 in range(QC):
                    w = slice(q2 * 512, (q2 + 1) * 512)
                    nc.tensor.matmul(
                        out=av_tile[:, w],
                        lhsT=vt8[:, kcp, :, head, 0:HD + 1],
                        rhs=E_tile[:, a, 2 * kcp:2 * kcp + 2, w],
                        start=start, stop=stop, perf_mode=DR,
                    )

            E_prev = None
            av0_prev = None

            for hp in range(4):
                E = epool.tile([128, 2, 8, HW], FP8, tag="E", name=f"E{hp}")
                av1 = ps_av.tile([65, HW], F32, tag="avA", bufs=1, name=f"av1_{hp}")
                av0_cur = None
                if hp == 3:
                    av0_cur = ps_av.tile([65, HW], F32, tag="avB", bufs=1, name="av0_3")
                for kc in range(8):
                    if hp == 0 and kc % 2 == 0:
                        vt_chunk(kc // 2)
                    psA = ps_s.tile([128, HW], F32, tag="ps_s", name=f"s{hp}_{kc}_1")
                    psB = ps_s.tile([128, HW], F32, tag="ps_s", name=f"s{hp}_{kc}_0")
                    for q2 in range(QC):
                        w = slice(q2 * 512, (q2 + 1) * 512)
                        nc.tensor.matmul(
                            out=psA[:, w],
                            lhsT=qk_sb[64:128, 4 + hp, kc * 128:(kc + 1) * 128],
                            rhs=qk_sb[64:128, hp, w],
                            start=True, stop=True,
                        )
                        nc.tensor.matmul(
                            out=psB[:, w],
                            lhsT=qk_sb[0:64, 4 + hp, kc * 128:(kc + 1) * 128],
                            rhs=qk_sb[0:64, hp, w],
                            start=True, stop=True,
                        )
                    nc.scalar.activation(out=E[:, 1, kc, :], in_=psA[:], func=AF.Exp, scale=SCALE, bias=ebias[:, 0:1])
                    nc.scalar.activation(out=E[:, 0, kc, :], in_=psB[:], func=AF.Exp, scale=SCALE, bias=ebias[:, 0:1])
                    # head a=1 AV: one DoubleRow pass per completed kc pair
                    if kc % 2 == 1:
                        av_dr(av1, 2 * hp + 1, E, 1, kc // 2, start=(kc == 1), stop=(kc == 7))
                    if hp < 3:
                        # previous pair's a=0 AV spread over kc 1..4
                        if E_prev is not None and 1 <= kc <= 4:
                            av_dr(av0_prev, 2 * (hp - 1), E_prev, 0, kc - 1,
                                  start=(kc == 1), stop=(kc == 4))
                            if kc == 4:
                                normalize_head(hp - 1, 0, av0_prev)
                        if kc == 4:
                            make_qk(hp + 1, ps_av, "avB")
                        if kc == 6:
                            make_qk(4 + hp + 1, ps_av, "avB")
                    else:
                        # last pair: finish pair 2's a=0 early, stream own a=0
                        if kc in (0, 1):
                            for bkcp in (2 * kc, 2 * kc + 1):
                                av_dr(av0_prev, 4, E_prev, 0, bkcp,
                                      start=(bkcp == 0), stop=(bkcp == 3))
                            if kc == 1:
                                normalize_head(2, 0, av0_prev)
                        if kc in (5, 7):
                            av_dr(av0_cur, 6, E, 0, kc // 2, start=(kc == 5), stop=False)
                if hp == 3:
                    for bkcp in (0, 1):
                        av_dr(av0_cur, 6, E, 0, bkcp, start=False, stop=(bkcp == 1))
                normalize_head(hp, 1, av1)
                if hp == 3:
                    normalize_head(3, 0, av0_cur)
                E_prev = E
                if hp < 3:
                    av0_prev = ps_av.tile([65, HW], F32, tag="avB", bufs=1, name=f"av0_{hp}")

            # ---- proj + residual tail ----
            def proj_mm(py, oc, cc):
                for q2 in range(QC):
                    nc.tensor.matmul(
                        out=py[:, q2 * 512:(q2 + 1) * 512],
                        lhsT=pw_sb[:, cc, oc * 128:(oc + 1) * 128],
                        rhs=att_t[cc][:, q2 * 512:(q2 + 1) * 512],
                        start=(cc == 0), stop=(cc == CC - 1),
                    )

            def proj_epilogue(py, oc):
                ot = outp.tile([128, HW], F32, tag="ot", name=f"ot{oc}")
                nc.vector.scalar_tensor_tensor(out=ot, in0=py[:], scalar=pb[:, oc:oc + 1], in1=x_sb[:, oc, :], op0=ALU.add, op1=ALU.add)
                nc.sync.dma_start(out=out_d.ap()[oc * 128:(oc + 1) * 128, :], in_=ot)

            pys = []
            for oc, (pool, tag) in enumerate(
                ((ps_s, "ps_s"), (ps_s, "ps_s"), (ps_av, "avA"), (ps_av, "avB"))
            ):
                py = pool.tile([128, HW], F32, tag=tag, bufs=None if tag == "ps_s" else 1, name=f"py{oc}")
                pys.append(py)
                for cc in range(CC - 1):
                    proj_mm(py, oc, cc)
            for oc in range(4):
                proj_mm(pys[oc], oc, CC - 1)
                proj_epilogue(pys[oc], oc)

    nc.compile()
    return nc


_NC_CACHE = None


def _get_nc():
    global _NC_CACHE
    if _NC_CACHE is None:
        _NC_CACHE = _build()
    return _NC_CACHE


def _prep_in_maps(inputs):
    x = np.asarray(inputs["x"], np.float32)
    gn_w = np.asarray(inputs["gn_w"], np.float32)
    gn_b = np.asarray(inputs["gn_b"], np.float32)
    qkv_w = np.asarray(inputs["qkv_w"], np.float32)
    qkv_b = np.asarray(inputs["qkv_b"], np.float32)
    proj_w = np.asarray(inputs["proj_w"], np.float32)
    proj_b = np.asarray(inputs["proj_b"], np.float32)

    B = x.shape[0]
    xr = x.reshape(B, C, HW).astype(ml_dtypes.bfloat16)
    qwT = np.ascontiguousarray(qkv_w.T).astype(ml_dtypes.bfloat16)
    pwT = np.ascontiguousarray(proj_w.T).astype(ml_dtypes.bfloat16)
    qkb = np.ascontiguousarray(qkv_b[: 2 * C].reshape(8, 128).T)
    vbb = np.ascontiguousarray(np.broadcast_to(qkv_b[2 * C:], (128, C)))
    pb = np.ascontiguousarray(proj_b.reshape(4, 128).T)
    gnw = np.ascontiguousarray(gn_w.reshape(4, 128).T)
    gnb = np.ascontiguousarray(gn_b.reshape(4, 128).T)
    indm = np.zeros((128, 8), np.float32)
    indm[np.arange(128), np.arange(128) // GSIZE] = 1.0 / GSIZE
    ind01 = np.zeros((128, 8), np.float32)
    ind01[np.arange(128), np.arange(128) // GSIZE] = 1.0
    indt = np.ascontiguousarray(ind01.T)
    shared = dict(qw=qwT, pw=pwT, qkb=qkb, vbb=vbb, pb=pb, gnw=gnw, gnb=gnb, ind=indm, indt=indt)
    return [dict(x=np.ascontiguousarray(xr[b]), **shared) for b in range(B)]


def kernel(**inputs):
    nc = _get_nc()
    in_maps = _prep_in_maps(inputs)
    res = run_bass_kernel_spmd(nc, in_maps, core_ids=list(range(8)))
    out = np.stack([r["out"] for r in res.results])
    return out.reshape(8, C, 32, 32).astype(np.float32)


def run_profiled(inputs):
    """kernel() + NTFF profiling; returns (output, exec_time_ns, trace_path)."""
    import types

    import antenv

    if "antenv.axon_hooks" not in sys.modules:
        hooks_mod = types.ModuleType("antenv.axon_hooks")
        _hook = [None]
        hooks_mod.set_axon_ntff_profile_hook = lambda h: _hook.__setitem__(0, h)
        hooks_mod.get_axon_ntff_profile_hook = lambda: _hook[0]
        sys.modules["antenv.axon_hooks"] = hooks_mod
        antenv.axon_hooks = hooks_mod
        from trn_agent_boot.trn_boot import _ntff_profile_via_ctypes

        hooks_mod.set_axon_ntff_profile_hook(_ntff_profile_via_ctypes("/opt/axon/libaxon_pjrt.so"))

    nc = _get_nc()
    in_maps = _prep_in_maps(inputs)
    res = run_bass_kernel_spmd(nc, in_maps, core_ids=list(range(8)), trace=True)
    out = np.stack([r["out"] for r in res.results]).reshape(8, C, 32, 32).astype(np.float32)
    trace = res.instructions_and_trace[1] if res.instructions_and_trace else None
    return out, res.exec_time_ns, trace


# revision 16
# speedup vs baseline: 1.3435x; 1.0239x over previous
"""AttentionBlock Trainium2 kernel: GroupNorm -> QKV -> MHA -> proj -> residual.

Data-parallel over batch B=8 across 8 NeuronCores (one batch image per core).
QKV/proj/score matmuls run in bf16 (fp32 PSUM accumulation); the
attention-value matmuls run in fp8 e4m3 DoubleRow (2 k-chunks per pass):
E is stored as exp(s*scale)/16 in e4m3 (the /16 keeps e4m3 in range; it
rides both the numerator and the ones-column denominator so it cancels).

Per-core layouts (C=512 channels, HW=1024 tokens, 8 heads, hd=64):
  x            [C, HW] bf16 (stats + residual tolerate bf16)
  xn           [C, HW] bf16, per-128-channel-chunk GroupNorm
  q, k         qk_sb[128, 8 oc, 1024] bf16; pair hp: oc=hp (q), 4+hp (k),
               head a at partitions a*64:(a+1)*64
  vT (fp8)     vt8[128, kcp 4, ko 2, head 8, 68] (65 used: 64 + ones col)
  scoresT      [k, q] PSUM [128, 1024] per (head, kc)
  E            [128, 2 head, 8 kc, 1024] fp8 per pair
  att          [C, HW] bf16; out [C, HW] fp32 = proj(att) + pb + x

PSUM (8 banks): ps_s x2 (scores / lead-in qk / proj py0-1) = 4 banks;
avA x1 (in-pair AV accumulator / py2) + avB x1 (rotating: GN stats, V^T,
mid-pair qk, deferred a=0 AV, py3) = 4 banks.

Softmax denominators: AV psum row 64 -> SBUF; reciprocal_approx_fast on the
row; DRAM bounce broadcast to [64, 1024]; multiply (partition-shifted write
for the odd head).
"""

import sys

if "/opt/trn_rl_repo" not in sys.path:
    sys.path.insert(0, "/opt/trn_rl_repo")

import numpy as np
import ml_dtypes

import concourse.bass as bass
import concourse.tile as tile
from concourse import mybir, bacc
from concourse.bass_utils import run_bass_kernel_spmd

AF = mybir.ActivationFunctionType
ALU = mybir.AluOpType
F32 = mybir.dt.float32
BF16 = mybir.dt.bfloat16
FP8 = mybir.dt.float8e4
DR = mybir.MatmulPerfMode.DoubleRow

C = 512
HW = 1024
NHEADS = 8
HD = 64
HDP = 68  # padded head stride so the DoubleRow ko-step (8*HDP) is 16-aligned
NGROUPS = 32
GSIZE = 16
EPS = 1e-5
SCALE = HD ** -0.5
EBIAS = -4.0 * float(np.log(2.0))  # exp output scaled by 1/16 for e4m3 range
CC = 4
OCQK = 8
QC = 2


def _build():
    nc = bacc.Bacc("TRN2", target_bir_lowering=False, debug=False, num_devices=8)

    x_d = nc.dram_tensor("x", [C, HW], BF16, kind="ExternalInput")
    qw_d = nc.dram_tensor("qw", [C, 3 * C], BF16, kind="ExternalInput")
    pw_d = nc.dram_tensor("pw", [C, C], BF16, kind="ExternalInput")
    qkb_d = nc.dram_tensor("qkb", [128, 8], F32, kind="ExternalInput")
    vbb_d = nc.dram_tensor("vbb", [128, C], F32, kind="ExternalInput")
    pb_d = nc.dram_tensor("pb", [128, 4], F32, kind="ExternalInput")
    gnw_d = nc.dram_tensor("gnw", [128, 4], F32, kind="ExternalInput")
    gnb_d = nc.dram_tensor("gnb", [128, 4], F32, kind="ExternalInput")
    ind_d = nc.dram_tensor("ind", [128, 8], F32, kind="ExternalInput")
    indt_d = nc.dram_tensor("indt", [8, 128], F32, kind="ExternalInput")
    out_d = nc.dram_tensor("out", [C, HW], F32, kind="ExternalOutput")

    with tile.TileContext(nc) as tc:
        with (
            tc.tile_pool(name="consts", bufs=1) as consts,
            tc.tile_pool(name="epool", bufs=2) as epool,
            tc.tile_pool(name="small", bufs=4) as small,
            tc.tile_pool(name="outp", bufs=3) as outp,
            tc.tile_pool(name="drp", bufs=4, space="DRAM") as drp,
            tc.tile_pool(name="ps_s", bufs=2, space="PSUM") as ps_s,
            tc.tile_pool(name="ps_av", bufs=1, space="PSUM") as ps_av,
        ):
            # ---- input DMAs: x chunks first (GN gates on them), then tables ----
            x_sb = consts.tile([128, CC, HW], BF16, tag="x")
            x_r = x_d.ap().rearrange("(cc p) hw -> p cc hw", p=128)
            for cc in range(CC):
                # alternate trigger queues so the per-trigger dispatch cost
                # (~0.7us on one sequencer) doesn't serialize the x chunks
                eng = nc.sync if cc % 2 == 0 else nc.scalar
                eng.dma_start(out=x_sb[:, cc, :], in_=x_r[:, cc, :])
            ind = consts.tile([128, 8], F32, tag="ind")
            nc.gpsimd.dma_start(out=ind, in_=ind_d.ap())
            indt = consts.tile([8, 128], F32, tag="indt")
            nc.gpsimd.dma_start(out=indt, in_=indt_d.ap())
            gnw = consts.tile([128, 4], F32, tag="gnw")
            nc.gpsimd.dma_start(out=gnw, in_=gnw_d.ap())
            gnb = consts.tile([128, 4], F32, tag="gnb")
            nc.gpsimd.dma_start(out=gnb, in_=gnb_d.ap())
            qkb = consts.tile([128, 8], F32, tag="qkb")
            nc.gpsimd.dma_start(out=qkb, in_=qkb_d.ap())
            vbb = consts.tile([128, C], F32, tag="vbb")
            nc.gpsimd.dma_start(out=vbb, in_=vbb_d.ap())
            pb = consts.tile([128, 4], F32, tag="pb")
            nc.gpsimd.dma_start(out=pb, in_=pb_d.ap())
            qw_sb = consts.tile([128, CC, 3 * C], BF16, tag="qw")
            qw_r = qw_d.ap().rearrange("(cc p) o -> p cc o", p=128)
            nc.sync.dma_start(out=qw_sb[:, :, 0:2 * C], in_=qw_r[:, :, 0:2 * C])
            nc.sync.dma_start(out=qw_sb[:, :, 2 * C:3 * C], in_=qw_r[:, :, 2 * C:3 * C])
            pw_sb = consts.tile([128, CC, C], BF16, tag="pw")
            nc.scalar.dma_start(out=pw_sb, in_=pw_d.ap().rearrange("(cc p) o -> p cc o", p=128))

            xn_sb = consts.tile([128, CC, HW], BF16, tag="xn")
            qk_sb = consts.tile([128, OCQK, HW], BF16, tag="qk")
            vt8 = consts.tile([128, 4, 2, NHEADS, HDP], FP8, tag="vt")
            att_t = [consts.tile([128, HW], BF16, tag=f"att{i}", name=f"att{i}") for i in range(CC)]

            # ones column of vT (softmax denominator trick)
            nc.vector.memset(vt8[:, :, :, :, HD:HD + 1], 1.0)
            # exp bias constant (E scaled by 1/16 for e4m3 range)
            ebias = consts.tile([128, 1], F32, tag="ebias")
            nc.vector.memset(ebias, EBIAS)

            # ---- GroupNorm: stats stream per chunk, then one batched chain ----
            st = small.tile([128, CC, 2, 6], F32, tag="gn_st")
            for cc in range(CC):
                nc.vector.bn_stats(out=st[:, cc, 0, :], in_=x_sb[:, cc, 0:512])
                nc.vector.bn_stats(out=st[:, cc, 1, :], in_=x_sb[:, cc, 512:1024])
            mv = small.tile([128, CC, 2], F32, tag="gn_mv")
            for cc in range(CC):
                nc.vector.bn_aggr(out=mv[:, cc, :], in_=st[:, cc, :, :])
            scr = small.tile([128, CC, 1], F32, tag="gn_scr")
            nc.vector.tensor_mul(out=scr, in0=mv[:, :, 0:1], in1=mv[:, :, 0:1])
            nc.vector.tensor_add(out=mv[:, :, 1:2], in0=mv[:, :, 1:2], in1=scr)
            pg = ps_s.tile([8, CC, 2], F32, tag="ps_s", name="gn_pg")
            nc.tensor.matmul(out=pg, lhsT=ind, rhs=mv.rearrange("p cc s -> p (cc s)"), start=True, stop=True)
            # sg: [mean, ex2->rstd] contiguous for the broadcast matmul; tt scratch
            sg = small.tile([8, CC, 2], F32, tag="gn_sg")
            tt = small.tile([8, CC, 2], F32, tag="gn_tt")
            nc.vector.tensor_copy(out=sg, in_=pg)
            nc.vector.scalar_tensor_tensor(out=tt[:, :, 0:1], in0=sg[:, :, 0:1], scalar=-1.0, in1=sg[:, :, 0:1], op0=ALU.mult, op1=ALU.mult)
            nc.vector.scalar_tensor_tensor(out=tt[:, :, 0:1], in0=sg[:, :, 1:2], scalar=EPS, in1=tt[:, :, 0:1], op0=ALU.add, op1=ALU.add)
            nc.scalar.activation(out=tt[:, :, 1:2], in_=tt[:, :, 0:1], func=AF.Sqrt, bias=0.0, scale=1.0)
            nc.vector.reciprocal(out=sg[:, :, 1:2], in_=tt[:, :, 1:2])
            pbc = ps_s.tile([128, CC, 2], F32, tag="ps_s", name="gn_pbc")
            nc.tensor.matmul(out=pbc, lhsT=indt, rhs=sg.rearrange("g cc s -> g (cc s)"), start=True, stop=True)
            ab = small.tile([128, CC, 2], F32, tag="gn_ab")
            nc.vector.tensor_mul(out=ab[:, :, 0:1], in0=pbc[:, :, 1:2], in1=gnw.rearrange("p (cc one) -> p cc one", one=1))
            nc.vector.scalar_tensor_tensor(out=ab[:, :, 1:2], in0=pbc[:, :, 0:1], scalar=-1.0, in1=ab[:, :, 0:1], op0=ALU.mult, op1=ALU.mult)
            nc.vector.tensor_add(out=ab[:, :, 1:2], in0=gnb.rearrange("p (cc one) -> p cc one", one=1), in1=ab[:, :, 1:2])
            for cc in range(CC):
                nc.vector.tensor_scalar(out=xn_sb[:, cc, :], in0=x_sb[:, cc, :], scalar1=ab[:, cc, 0:1], scalar2=ab[:, cc, 1:2], op0=ALU.mult, op1=ALU.add)

            # ---- q/k production ----
            def make_qk(oc, pool, tag, bias_on_scalar=False):
                pq = pool.tile([128, HW], F32, tag=tag, bufs=None if tag == "ps_s" else 1, name=f"pq{oc}")
                for cc in range(CC):
                    for q2 in range(QC):
                        nc.tensor.matmul(
                            out=pq[:, q2 * 512:(q2 + 1) * 512],
                            lhsT=qw_sb[:, cc, oc * 128:(oc + 1) * 128],
                            rhs=xn_sb[:, cc, q2 * 512:(q2 + 1) * 512],
                            start=(cc == 0), stop=(cc == CC - 1),
                        )
                if bias_on_scalar:
                    nc.scalar.add(out=qk_sb[:, oc, :], in_=pq[:], add=qkb[:, oc:oc + 1])
                else:
                    nc.vector.tensor_scalar_add(out=qk_sb[:, oc, :], in0=pq[:], scalar1=qkb[:, oc:oc + 1])

            make_qk(0, ps_s, "ps_s", bias_on_scalar=True)
            make_qk(4, ps_s, "ps_s", bias_on_scalar=True)

            # ---- V^T chunks (2 hw-chunks per psum tile), woven into pair 0 ----
            def vt_chunk(i):
                pv = ps_av.tile([128, 2, 512], F32, tag="avB", bufs=1, name=f"pv{i}")
                for h2 in range(2):
                    hwc = 2 * i + h2
                    for cc in range(CC):
                        nc.tensor.matmul(
                            out=pv[:, h2, :],
                            lhsT=xn_sb[:, cc, hwc * 128:(hwc + 1) * 128],
                            rhs=qw_sb[:, cc, 2 * C:3 * C],
                            start=(cc == 0), stop=(cc == CC - 1),
                        )
                for h2 in range(2):
                    hwc = 2 * i + h2
                    nc.vector.tensor_add(
                        out=vt8[:, hwc // 2, hwc % 2, :, 0:HD],
                        in0=pv[:, h2, :].rearrange("p (h d) -> p h d", d=HD),
                        in1=vbb[:].rearrange("p (h d) -> p h d", d=HD),
                    )

            # ---- attention ----
            def normalize_head(hp, a, av_tile):
                avs = small.tile([65, HW], F32, tag="avs", name=f"avs{hp}_{a}")
                nc.vector.tensor_copy(out=avs, in_=av_tile[:, :])
                # denominator row -> [128, 8] (direct SBUF reshape DMA) so the
                # reciprocal runs across all partitions, then DRAM-bounce the
                # bf16 reciprocal out to a [64, 1024] broadcast
                eng = nc.gpsimd if a == 1 else nc.sync
                dt = small.tile([128, 8], F32, tag="dt", name=f"dt{hp}_{a}")
                eng.dma_start(out=dt, in_=avs[64:65, :])
                dtb = small.tile([128, 8], BF16, tag="dtb", name=f"dtb{hp}_{a}")
                with nc.allow_low_precision(reason="bf16 softmax denominator broadcast"):
                    nc.vector.reciprocal(out=dtb, in_=dt)
                rr = drp.tile([HW], BF16, tag="rr", name=f"rr{hp}_{a}")
                eng.dma_start(out=rr, in_=dtb)
                sbc = small.tile([64, HW], BF16, tag="sbc", name=f"sbc{hp}_{a}")
                eng.dma_start(
                    out=sbc,
                    in_=bass.AP(tensor=rr.tensor, offset=rr.offset, ap=[[0, 64]] + list(rr.ap)),
                )
                # partition-shifted DVE write places the odd head at 64:128
                nc.vector.tensor_mul(out=att_t[hp][a * 64:(a + 1) * 64, :], in0=avs[0:64, :], in1=sbc)

            def av_dr(av_tile, head, E_tile, a, kcp, start, stop):
                # fp8 DoubleRow: contracts k-chunks 2*kcp and 2*kcp+1 in one pass
                for q2 in range(QC):
                    w = slice(q2 * 512, (q2 + 1) * 512)
                    nc.tensor.matmul(
                        out=av_tile[:, w],
                        lhsT=vt8[:, kcp, :, head, 0:HD + 1],
                        rhs=E_tile[:, a, 2 * kcp:2 * kcp + 2, w],
                        start=start, stop=stop, perf_mode=DR,
                    )

            E_prev = None
            av0_prev = None

            for hp in range(4):
                E = epool.tile([128, 2, 8, HW], FP8, tag="E", name=f"E{hp}")
                av1 = ps_av.tile([65, HW], F32, tag="avA", bufs=1, name=f"av1_{hp}")
                av0_cur = None
                if hp == 3:
                    av0_cur = ps_av.tile([65, HW], F32, tag="avB", bufs=1, name="av0_3")
                for kc in range(8):
                    if hp == 0 and kc % 2 == 0:
                        vt_chunk(kc // 2)
                    psA = ps_s.tile([128, HW], F32, tag="ps_s", name=f"s{hp}_{kc}_1")
                    psB = ps_s.tile([128, HW], F32, tag="ps_s", name=f"s{hp}_{kc}_0")
                    for q2 in range(QC):
                        w = slice(q2 * 512, (q2 + 1) * 512)
                        nc.tensor.matmul(
                            out=psA[:, w],
                            lhsT=qk_sb[64:128, 4 + hp, kc * 128:(kc + 1) * 128],
                            rhs=qk_sb[64:128, hp, w],
                            start=True, stop=True,
                        )
                        nc.tensor.matmul(
                            out=psB[:, w],
                            lhsT=qk_sb[0:64, 4 + hp, kc * 128:(kc + 1) * 128],
                            rhs=qk_sb[0:64, hp, w],
                            start=True, stop=True,
                        )
                    nc.scalar.activation(out=E[:, 1, kc, :], in_=psA[:], func=AF.Exp, scale=SCALE, bias=ebias[:, 0:1])
                    nc.scalar.activation(out=E[:, 0, kc, :], in_=psB[:], func=AF.Exp, scale=SCALE, bias=ebias[:, 0:1])
                    # head a=1 AV: one DoubleRow pass per completed kc pair
                    if kc % 2 == 1:
                        av_dr(av1, 2 * hp + 1, E, 1, kc // 2, start=(kc == 1), stop=(kc == 7))
                    if hp < 3:
                        # previous pair's a=0 AV spread over kc 1..4
                        if E_prev is not None and 1 <= kc <= 4:
                            av_dr(av0_prev, 2 * (hp - 1), E_prev, 0, kc - 1,
                                  start=(kc == 1), stop=(kc == 4))
                            if kc == 4:
                                normalize_head(hp - 1, 0, av0_prev)
                        if kc == 4:
                            make_qk(hp + 1, ps_av, "avB")
                        if kc == 6:
                            make_qk(4 + hp + 1, ps_av, "avB")
                    else:
                        # last pair: finish pair 2's a=0 early, stream own a=0
                        if kc in (0, 1):
                            for bkcp in (2 * kc, 2 * kc + 1):
                                av_dr(av0_prev, 4, E_prev, 0, bkcp,
                                      start=(bkcp == 0), stop=(bkcp == 3))
                            if kc == 1:
                                normalize_head(2, 0, av0_prev)
                        if kc in (5, 7):
                            av_dr(av0_cur, 6, E, 0, kc // 2, start=(kc == 5), stop=False)
                if hp == 3:
                    for bkcp in (0, 1):
                        av_dr(av0_cur, 6, E, 0, bkcp, start=False, stop=(bkcp == 1))
                normalize_head(hp, 1, av1)
                if hp == 3:
                    normalize_head(3, 0, av0_cur)
                E_prev = E
                if hp < 3:
                    av0_prev = ps_av.tile([65, HW], F32, tag="avB", bufs=1, name=f"av0_{hp}")

            # ---- proj + residual tail ----
            def proj_mm(py, oc, cc):
                for q2 in range(QC):
                    nc.tensor.matmul(
                        out=py[:, q2 * 512:(q2 + 1) * 512],
                        lhsT=pw_sb[:, cc, oc * 128:(oc + 1) * 128],
                        rhs=att_t[cc][:, q2 * 512:(q2 + 1) * 512],
                        start=(cc == 0), stop=(cc == CC - 1),
                    )

            def proj_epilogue(py, oc):
                ot = outp.tile([128, HW], F32, tag="ot", name=f"ot{oc}")
                nc.vector.scalar_tensor_tensor(out=ot, in0=py[:], scalar=pb[:, oc:oc + 1], in1=x_sb[:, oc, :], op0=ALU.add, op1=ALU.add)
                nc.sync.dma_start(out=out_d.ap()[oc * 128:(oc + 1) * 128, :], in_=ot)

            pys = []
            for oc, (pool, tag) in enumerate(
                ((ps_s, "ps_s"), (ps_s, "ps_s"), (ps_av, "avA"), (ps_av, "avB"))
            ):
                py = pool.tile([128, HW], F32, tag=tag, bufs=None if tag == "ps_s" else 1, name=f"py{oc}")
                pys.append(py)
                for cc in range(CC - 1):
                    proj_mm(py, oc, cc)
            for oc in range(4):
                proj_mm(pys[oc], oc, CC - 1)
                proj_epilogue(pys[oc], oc)

    nc.compile()
    return nc


_NC_CACHE = None


def _get_nc():
    global _NC_CACHE
    if _NC_CACHE is None:
        _NC_CACHE = _build()
    return _NC_CACHE


def _prep_in_maps(inputs):
    x = np.asarray(inputs["x"], np.float32)
    gn_w = np.asarray(inputs["gn_w"], np.float32)
    gn_b = np.asarray(inputs["gn_b"], np.float32)
    qkv_w = np.asarray(inputs["qkv_w"], np.float32)
    qkv_b = np.asarray(inputs["qkv_b"], np.float32)
    proj_w = np.asarray(inputs["proj_w"], np.float32)
    proj_b = np.asarray(inputs["proj_b"], np.float32)

    B = x.shape[0]
    xr = x.reshape(B, C, HW).astype(ml_dtypes.bfloat16)
    qwT = np.ascontiguousarray(qkv_w.T).astype(ml_dtypes.bfloat16)
    pwT = np.ascontiguousarray(proj_w.T).astype(ml_dtypes.bfloat16)
    qkb = np.ascontiguousarray(qkv_b[: 2 * C].reshape(8, 128).T)
    vbb = np.ascontiguousarray(np.broadcast_to(qkv_b[2 * C:], (128, C)))
    pb = np.ascontiguousarray(proj_b.reshape(4, 128).T)
    gnw = np.ascontiguousarray(gn_w.reshape(4, 128).T)
    gnb = np.ascontiguousarray(gn_b.reshape(4, 128).T)
    indm = np.zeros((128, 8), np.float32)
    indm[np.arange(128), np.arange(128) // GSIZE] = 1.0 / GSIZE
    ind01 = np.zeros((128, 8), np.float32)
    ind01[np.arange(128), np.arange(128) // GSIZE] = 1.0
    indt = np.ascontiguousarray(ind01.T)
    shared = dict(qw=qwT, pw=pwT, qkb=qkb, vbb=vbb, pb=pb, gnw=gnw, gnb=gnb, ind=indm, indt=indt)
    return [dict(x=np.ascontiguousarray(xr[b]), **shared) for b in range(B)]


def kernel(**inputs):
    nc = _get_nc()
    in_maps = _prep_in_maps(inputs)
    res = run_bass_kernel_spmd(nc, in_maps, core_ids=list(range(8)))
    out = np.stack([r["out"] for r in res.results])
    return out.reshape(8, C, 32, 32).astype(np.float32)


def run_profiled(inputs):
    """kernel() + NTFF profiling; returns (output, exec_time_ns, trace_path)."""
    import types

    import antenv

    if "antenv.axon_hooks" not in sys.modules:
        hooks_mod = types.ModuleType("antenv.axon_hooks")
        _hook = [None]
        hooks_mod.set_axon_ntff_profile_hook = lambda h: _hook.__setitem__(0, h)
        hooks_mod.get_axon_ntff_profile_hook = lambda: _hook[0]
        sys.modules["antenv.axon_hooks"] = hooks_mod
        antenv.axon_hooks = hooks_mod
        from trn_agent_boot.trn_boot import _ntff_profile_via_ctypes

        hooks_mod.set_axon_ntff_profile_hook(_ntff_profile_via_ctypes("/opt/axon/libaxon_pjrt.so"))

    nc = _get_nc()
    in_maps = _prep_in_maps(inputs)
    res = run_bass_kernel_spmd(nc, in_maps, core_ids=list(range(8)), trace=True)
    out = np.stack([r["out"] for r in res.results]).reshape(8, C, 32, 32).astype(np.float32)
    trace = res.instructions_and_trace[1] if res.instructions_and_trace else None
    return out, res.exec_time_ns, trace
